# revision 9
# baseline (speedup 1.0000x reference)
"""AttentionWithRoPE on 8 Trainium2 NeuronCores.

Sharding: batch x query-half -> 8 independent cores (no collectives).
Core c handles batch b=c//2, query rows [qh*1024, (qh+1)*1024) with qh=c%2.
The host rolls the t axis per core so the query rows always sit in columns
[0, TQ) of xT; cosk/sink are rolled identically, so RoPE sees true positions
and the softmax key set is unchanged (order-invariant).

Per-core plan:
  Inputs x/Wq/Wk/Wv/Wp arrive bf16 (host-converted); V/attnout staging and
  the softmax P tiles are bf16; S = K^T.T@Q^T runs as an fp8e4 DoubleRow
  matmul (2x PE rate, head-dim packed [32,2] via SBUF repack DMAs); the
  final out projection accumulates in fp32 PSUM and is emitted fp32.
  V:    V projection in natural layout [t, j], staged to DRAM per-head
        contiguous (Vd3[h] is a ready-to-load [128, 16, 64] bf16 block; a
        ones column is appended in SBUF for softmax row sums).
  QK+attention, software-pipelined per head pair p:
        Q^T/K^T projections for pair p+1 (bf16, RoPE via signed pair-swap
        permutation matmul + DVE/GPSIMD combine, then cast-repacked to the
        fp8 [32,2,t] DoubleRow layout) are emitted interleaved with
        attention for pair p so the PE fills the gaps while ACT grinds
        through the softmax exps.
        Attention per head: S^T[t,l] via fp8 DoubleRow, P^T = exp(scale*S^T)
        on ACT in bf16 (logits are O(1), no max subtraction), attnout^T =
        [V_h|1].T @ P^T accumulated over t which also yields the softmax
        row sums in partition 64; normalize via reciprocal + GPSIMD
        partition-broadcast, then stage normalized attnout^T (bf16) to DRAM.
  Out:  out = attnout^T.T @ Wp + bp in fp32.
"""

import sys

sys.path.insert(0, "/opt/trn_rl_repo")

from contextlib import ExitStack

import ml_dtypes
import numpy as np

import concourse.bass as bass
import concourse.mybir as mybir
import concourse.tile as tile
from concourse.bass_utils import run_bass_kernel_spmd

F32 = mybir.dt.float32
F32R = mybir.dt.float32r
BF16 = mybir.dt.bfloat16
F8 = mybir.dt.float8e4
AF = mybir.ActivationFunctionType
MUL = mybir.AluOpType.mult
DR = mybir.MatmulPerfMode.DoubleRow

B, T, D = 4, 2048, 1024
H, HD = 16, 64
P = 128
TQ = 1024  # query rows per core
SCALE = float(D) ** -0.5
ROPE_THETA = 10000.0

_ws_ctr = [0]


def _split_multi_waits(nc):
    """The walrus build in this container accepts at most one sync-wait per
    engine instruction. Hoist all but one wait of each instruction into
    standalone EventSemaphore instructions on the same engine, placed
    immediately before it (engines are in-order, so semantics are identical)."""
    n = 0
    for f in nc.m.functions:
        for blk in f.blocks:
            insts = list(blk.instructions)
            newlist = []
            changed = False
            for inst in insts:
                si = getattr(inst, "sync_info", None)
                waits = list(si.on_wait) if si is not None and si.on_wait else []
                if len(waits) > 1:
                    for w in waits[:-1]:
                        _ws_ctr[0] += 1
                        evs = mybir.InstEventSemaphore(
                            name=f"WSPLIT-{_ws_ctr[0]}", ins=[], outs=[]
                        )
                        evs.engine = inst.engine
                        evs.sync_info = mybir.SyncInfo(on_wait=[w], on_update=[])
                        newlist.append(evs)
                        n += 1
                    inst.sync_info = mybir.SyncInfo(
                        on_wait=[waits[-1]], on_update=list(si.on_update)
                    )
                    changed = True
                newlist.append(inst)
            if changed:
                blk.instructions[:] = newlist
    return n


def _perm_lhsT():
    """lhsT for the rotate-half permutation: out = lhsT.T @ q gives
    out[2i] = -q[2i+1], out[2i+1] = q[2i]."""
    m = np.zeros((P, P), np.float32)
    for i in range(P // 2):
        m[2 * i + 1, 2 * i] = -1.0
        m[2 * i, 2 * i + 1] = 1.0
    return m


def build_nc(split_waits=True, reps=1):
    nc = bass.Bass(trn_type="TRN2", target_bir_lowering=False, debug=False)

    xT = nc.dram_tensor("xT", [D, T], BF16, kind="ExternalInput").ap()
    Wq = nc.dram_tensor("Wq", [D, D], BF16, kind="ExternalInput").ap()
    Wk = nc.dram_tensor("Wk", [D, D], BF16, kind="ExternalInput").ap()
    Wv = nc.dram_tensor("Wv", [D, D], BF16, kind="ExternalInput").ap()
    Wp = nc.dram_tensor("Wp", [D, D], BF16, kind="ExternalInput").ap()
    bpb = nc.dram_tensor("bpb", [P, D], F32, kind="ExternalInput").ap()
    cosk = nc.dram_tensor("cosk", [P, T], F32, kind="ExternalInput").ap()
    sink = nc.dram_tensor("sink", [P, T], F32, kind="ExternalInput").ap()
    out = nc.dram_tensor("out", [TQ, D], F32, kind="ExternalOutput").ap()

    permc = nc.inline_tensor(_perm_lhsT().astype(ml_dtypes.bfloat16), name="permc").ap()
    ones128c = nc.inline_tensor(np.ones((P, 64), np.float32), name="ones128c").ap()
    onescolc = nc.inline_tensor(
        np.ones((P, T // P, 1), ml_dtypes.bfloat16), name="onescolc"
    ).ap()

    # Vd3[h] is a contiguous [128(tp), 16(to), 64] bf16 per-head V block.
    # AOTd holds normalized attnout^T in bf16.
    Vd3 = nc.dram_tensor("Vd3", [H, P, T // P, HD], BF16).ap()
    AOTd = nc.dram_tensor("AOTd", [D // P, P, TQ], BF16).ap()

    xT_r = xT.rearrange("(do dp) t -> dp do t", dp=P)  # [128, 8, 2048]
    Wq_r = Wq.rearrange("(do dp) j -> dp do j", dp=P)
    Wk_r = Wk.rearrange("(do dp) j -> dp do j", dp=P)
    Wv_r = Wv.rearrange("(do dp) j -> dp do j", dp=P)
    Wp_r = Wp.rearrange("(ko kp) j -> kp ko j", kp=P)
    AOTd_r = AOTd.rearrange("ko p l -> p ko l")  # [128, 8, 1024]

    DO = D // P  # 8 contraction tiles
    NT = T // P  # 16 key tiles

    with tile.TileContext(nc) as tc:
      for _rep in range(reps):
        with ExitStack() as top:
            persist = top.enter_context(tc.tile_pool(name="persist", bufs=1))
            ones128 = persist.tile([P, 64], F32R)
            onesva = persist.tile([P, NT, 1], BF16)
            permt = persist.tile([P, P], BF16)
            ck = persist.tile([P, T], F32)
            sk = persist.tile([P, T], F32)

            with tc.tile_pool(name="xpool", bufs=1) as xpool:
                xts = xpool.tile([P, DO, T], BF16)

                # -------- interleaved Q/K projection + attention pipeline ------
                with ExitStack() as pmain:
                    wv1pool = pmain.enter_context(tc.tile_pool(name="w_v1", bufs=1))
                    vpool = pmain.enter_context(tc.tile_pool(name="o_v1", bufs=2))
                    wpool = pmain.enter_context(tc.tile_pool(name="w_qk", bufs=2))
                    tpool = pmain.enter_context(tc.tile_pool(name="t_qk", bufs=2))
                    kqpool = pmain.enter_context(tc.tile_pool(name="kq", bufs=2))
                    k8pool = pmain.enter_context(tc.tile_pool(name="k8", bufs=2))
                    pmm = pmain.enter_context(
                        tc.tile_pool(name="ps_qk", bufs=1, space="PSUM")
                    )
                    pmisc = pmain.enter_context(
                        tc.tile_pool(name="ps_misc", bufs=1, space="PSUM")
                    )

                    def proj_rope(w_t, dst, dst_col, src_col):
                        """One [128, 512] Q/K projection tile + RoPE into dst."""
                        ps = pmm.tile([P, 512], F32, tag="ps")
                        for do in range(DO):
                            nc.tensor.matmul(
                                ps[:],
                                w_t[:, do],
                                xts[:, do, src_col * 512 : (src_col + 1) * 512],
                                start=(do == 0),
                                stop=(do == DO - 1),
                            )
                        raw = tpool.tile([P, 512], BF16, tag="raw")
                        nc.vector.tensor_copy(raw[:], ps[:])
                        pr = pmisc.tile([P, 512], F32, tag="misc")
                        nc.tensor.matmul(pr[:], permt[:], raw[:], start=True, stop=True)
                        t1 = tpool.tile([P, 512], BF16, tag="t1")
                        nc.gpsimd.tensor_tensor(
                            t1[:],
                            raw[:],
                            ck[:, src_col * 512 : (src_col + 1) * 512],
                            MUL,
                        )
                        t2 = tpool.tile([P, 512], BF16, tag="t2")
                        nc.vector.tensor_mul(
                            t2[:], pr[:], sk[:, src_col * 512 : (src_col + 1) * 512]
                        )
                        nc.vector.tensor_add(
                            dst[:, dst_col * 512 : (dst_col + 1) * 512], t1[:], t2[:]
                        )

                    def emit_proj_jo(jo, preload=False):
                        """Q^T/K^T for head pair jo: 6 projection-tile closures
                        + 1 fp8 repack closure; returns the packed fp8 tiles."""
                        ktp = kqpool.tile([P, T], F8, tag="ktp")
                        qtp = kqpool.tile([P, TQ], F8, tag="qtp")
                        kp8 = k8pool.tile([P, 2, T], F8, tag="kp8")
                        qp8 = k8pool.tile([P, 2, TQ], F8, tag="qp8")
                        steps = []

                        def load_w():
                            wq_t = wpool.tile([P, DO, P], BF16, tag="wq")
                            nc.sync.dma_start(
                                wq_t[:], Wq_r[:, :, jo * P : (jo + 1) * P]
                            )
                            wk_t = wpool.tile([P, DO, P], BF16, tag="wk")
                            nc.sync.dma_start(
                                wk_t[:], Wk_r[:, :, jo * P : (jo + 1) * P]
                            )
                            return wq_t, wk_t

                        wref = []
                        if preload:
                            wref.extend(load_w())

                        def step(i):
                            if i == 0 and not wref:
                                wref.extend(load_w())
                            wq_t, wk_t = wref
                            if i < 2:
                                proj_rope(wq_t, qtp, i, i)
                            else:
                                proj_rope(wk_t, ktp, i - 2, i - 2)

                        def repack():
                            # head-dim [32,2] packing for the DoubleRow S
                            # matmul (pure fp8 moves on the HWDGE path).
                            for h in range(2):
                                for i in range(2):
                                    src = ktp[64 * h + 32 * i : 64 * h + 32 * i + 32, :]
                                    nc.sync.dma_start(
                                        kp8[64 * h : 64 * h + 32, i, :], src
                                    )
                                    srcq = qtp[64 * h + 32 * i : 64 * h + 32 * i + 32, :]
                                    nc.sync.dma_start(
                                        qp8[64 * h : 64 * h + 32, i, :], srcq
                                    )

                        for i in range(6):
                            steps.append(lambda i=i: step(i))
                        steps.append(repack)
                        return kp8, qp8, steps

                    # ------- V first half (heads 0-7) + jo0 proj interleaved ----
                    with ExitStack() as p1b:
                        wvpool = p1b.enter_context(tc.tile_pool(name="w_v", bufs=1))
                        vpool0 = p1b.enter_context(tc.tile_pool(name="o_v", bufs=2))
                        pmmv = p1b.enter_context(
                            tc.tile_pool(name="ps_v", bufs=2, space="PSUM")
                        )
                        wv_t0 = wvpool.tile([P, DO, 512], BF16, tag="wv")
                        nc.sync.dma_start(wv_t0[:], Wv_r[:, :, 0:512])
                        kp80, qp80, steps0 = emit_proj_jo(0, preload=True)
                        for q in range(8):
                            eng = nc.sync if q % 2 == 0 else nc.gpsimd
                            eng.dma_start(
                                xts[:, :, q * 256 : (q + 1) * 256],
                                xT_r[:, :, q * 256 : (q + 1) * 256],
                            )
                        nc.gpsimd.dma_start(ones128[:], ones128c.bitcast(F32R))
                        nc.gpsimd.dma_start(onesva[:], onescolc)
                        nc.gpsimd.dma_start(permt[:], permc)
                        nc.gpsimd.dma_start(ck[:], cosk[:])
                        nc.gpsimd.dma_start(sk[:], sink[:])
                        jo0_slots = {3: 0, 5: 1, 7: 2, 9: 3, 11: 4, 13: 5, 15: 6}
                        for to in range(NT):
                            ps = pmmv.tile([P, 512], F32, tag="psv")
                            for do in range(DO):
                                nc.tensor.matmul(
                                    ps[:],
                                    xts[:, do, to * P : (to + 1) * P],
                                    wv_t0[:, do],
                                    start=(do == 0),
                                    stop=(do == DO - 1),
                                )
                            vt = vpool0.tile([P, 512], BF16, tag="vt")
                            nc.vector.tensor_copy(vt[:], ps[:])
                            nc.sync.dma_start(
                                Vd3[0:8, :, to, :].rearrange("h tp e -> tp h e"),
                                vt.rearrange("tp (h e) -> tp h e", e=64),
                            )
                            if to in jo0_slots:
                                steps0[jo0_slots[to]]()

                    vapool = pmain.enter_context(tc.tile_pool(name="va", bufs=2))
                    ptpool = pmain.enter_context(tc.tile_pool(name="pt", bufs=4))
                    smpool = pmain.enter_context(tc.tile_pool(name="sm", bufs=2))
                    pss = pmain.enter_context(
                        tc.tile_pool(name="ps_s", bufs=2, space="PSUM")
                    )
                    pso = pmain.enter_context(
                        tc.tile_pool(name="ps_o", bufs=2, space="PSUM")
                    )

                    wv1 = wv1pool.tile([P, DO, 512], BF16, tag="wv1")
                    nc.sync.dma_start(wv1[:], Wv_r[:, :, 512:1024])

                    def v_jc1_step(to):
                        ps = pmm.tile([P, 512], F32, tag="ps", name="psv1")
                        for do in range(DO):
                            nc.tensor.matmul(
                                ps[:],
                                xts[:, do, to * P : (to + 1) * P],
                                wv1[:, do],
                                start=(do == 0),
                                stop=(do == DO - 1),
                            )
                        vt = vpool.tile([P, 512], BF16, tag="vt1")
                        nc.vector.tensor_copy(vt[:], ps[:])
                        nc.sync.dma_start(
                            Vd3[8:16, :, to, :].rearrange("h tp e -> tp h e"),
                            vt.rearrange("tp (h e) -> tp h e", e=64),
                        )

                    def emit_attention(p, kp8, qp8, interleave):
                        """Attention for head pair p. `interleave` is a list of
                        closures (next pair's projection steps) sprinkled between
                        blocks to keep the PE busy while ACT runs the exps."""
                        va = vapool.tile([P, NT, 65], BF16, tag="va")
                        nc.sync.dma_start(va[:, :, 0:64], Vd3[2 * p])
                        nc.vector.tensor_copy(va[:, :, 64:65], onesva[:])
                        vb = vapool.tile([P, NT, 65], BF16, tag="vb")
                        nc.sync.dma_start(vb[:, :, 0:64], Vd3[2 * p + 1])
                        nc.vector.tensor_copy(vb[:, :, 64:65], onesva[:])

                        il = list(interleave)
                        n_it = 2 * NT
                        il_at = {}
                        for i in range(len(il)):
                            slot = 1 + (i * (n_it - 4)) // max(1, len(il))
                            while slot in il_at:
                                slot += 1
                            il_at[slot] = i
                        it_ctr = [0]

                        for lc in range(TQ // 512):
                            pos = [
                                pso.tile([P, 512], F32, tag=f"po{h}", name=f"po{h}")
                                for h in range(2)
                            ]
                            vvs = [va, vb]
                            for to in range(NT):
                                pts = []
                                for h in range(2):
                                    ps = pss.tile([P, 512], F32, tag="pss")
                                    nc.tensor.matmul(
                                        ps[:],
                                        kp8[64 * h : 64 * h + 32, :,
                                            to * P : (to + 1) * P],
                                        qp8[64 * h : 64 * h + 32, :,
                                            lc * 512 : (lc + 1) * 512],
                                        start=True,
                                        stop=True,
                                        perf_mode=DR,
                                    )
                                    pt = ptpool.tile([P, 512], BF16, tag="pt")
                                    nc.scalar.activation(
                                        pt[:], ps[:], AF.Exp, scale=SCALE
                                    )
                                    pts.append(pt)
                                for h in range(2):
                                    nc.tensor.matmul(
                                        pos[h][0:65, :],
                                        vvs[h][:, to, :],
                                        pts[h][:],
                                        start=(to == 0),
                                        stop=(to == NT - 1),
                                    )
                                k = it_ctr[0]
                                it_ctr[0] += 1
                                if k in il_at and il_at[k] < len(il):
                                    il[il_at[k]]()
                            for h in range(2):
                                po = pos[h]
                                # copy attnout^T + row sums out of PSUM, then
                                # normalize: row 64 holds the softmax sums
                                po_s = smpool.tile([65, 512], F32, tag="pos")
                                nc.vector.tensor_copy(po_s[:], po[0:65, :])
                                rc = smpool.tile([65, 512], F32R, tag="rc")
                                with nc.allow_low_precision(
                                    reason="f32r feeds the partition broadcast"
                                ):
                                    nc.vector.reciprocal(
                                        rc[64:65, :], po_s[64:65, :]
                                    )
                                pb = pmisc.tile([P, 512], F32, tag="misc", name="pb")[0:64, :]
                                nc.tensor.matmul(
                                    pb[:],
                                    ones128[64:65, :],
                                    rc[64:65, :],
                                    start=True,
                                    stop=True,
                                )
                                rbb = smpool.tile([64, 512], F32, tag="rbb")
                                nc.vector.tensor_copy(rbb[:], pb[:])
                                tmpn = smpool.tile([64, 512], BF16, tag="tmpn")
                                nc.vector.tensor_mul(tmpn[:], po_s[0:64, :], rbb[:])
                                nc.sync.dma_start(
                                    AOTd[p, 64 * h : 64 * h + 64,
                                         lc * 512 : (lc + 1) * 512],
                                    tmpn[:],
                                )
                        done = {il_at[k] for k in il_at if il_at[k] < len(il)}
                        for i in range(len(il)):
                            if i not in done:
                                il[i]()

                    kp8c, qp8c = kp80, qp80
                    for p in range(DO):
                        if p + 1 < DO:
                            nkp8, nqp8, nsteps = emit_proj_jo(p + 1)
                        else:
                            nkp8 = nqp8 = None
                            nsteps = []
                        if p == 0:
                            nsteps = nsteps + [
                                (lambda to=to: v_jc1_step(to)) for to in range(6)
                            ]
                        elif p == 1:
                            nsteps = nsteps + [
                                (lambda to=to: v_jc1_step(to)) for to in range(6, 11)
                            ]
                        elif p == 2:
                            nsteps = nsteps + [
                                (lambda to=to: v_jc1_step(to)) for to in range(11, NT)
                            ]
                        emit_attention(p, kp8c, qp8c, nsteps)
                        kp8c, qp8c = nkp8, nqp8

            # ---------------- output projection -------------------------------
            with ExitStack() as p3:
                wppool = p3.enter_context(tc.tile_pool(name="wp", bufs=1))
                wp_t = wppool.tile([P, DO, D], BF16)
                for ko in range(DO):
                    nc.sync.dma_start(wp_t[:, ko], Wp_r[:, ko])
                bpt = wppool.tile([P, D], F32)
                nc.sync.dma_start(bpt[:], bpb[:])
                apool = p3.enter_context(tc.tile_pool(name="aot", bufs=3))
                outpool = p3.enter_context(tc.tile_pool(name="outp", bufs=3))
                ps_f = p3.enter_context(
                    tc.tile_pool(name="ps_f", bufs=4, space="PSUM")
                )
                for lt in range(TQ // P):
                    aot_t = apool.tile([P, DO, P], BF16, tag="aot")
                    nc.sync.dma_start(
                        aot_t[:], AOTd_r[:, :, lt * P : (lt + 1) * P]
                    )
                    for jc in range(2):
                        ps = ps_f.tile([P, 512], F32, tag="psf")
                        for ko in range(DO):
                            nc.tensor.matmul(
                                ps[:],
                                aot_t[:, ko],
                                wp_t[:, ko, jc * 512 : (jc + 1) * 512],
                                start=(ko == 0),
                                stop=(ko == DO - 1),
                            )
                        ot = outpool.tile([P, 512], F32, tag="oto")
                        nc.vector.tensor_add(
                            ot[:], ps[:], bpt[:, jc * 512 : (jc + 1) * 512]
                        )
                        nc.sync.dma_start(
                            out[lt * P : (lt + 1) * P, jc * 512 : (jc + 1) * 512],
                            ot[:],
                        )

    if split_waits:
        _split_multi_waits(nc)
    return nc


def _rope_tables():
    inv = 1.0 / (ROPE_THETA ** (np.arange(0, HD, 2, dtype=np.float32) / HD))
    t = np.arange(T, dtype=np.float32)
    freqs = np.einsum("i,j->ij", t, inv)  # [T, 32]
    freqs = np.repeat(freqs, 2, axis=-1)  # [T, 64]
    cosT = np.cos(freqs).T  # [64, T]
    sinT = np.sin(freqs).T
    cosk = np.tile(cosT, (2, 1)).astype(np.float32)  # [128, T]
    sink = np.tile(sinT, (2, 1)).astype(np.float32)
    return np.ascontiguousarray(cosk), np.ascontiguousarray(sink)


_NC_CACHE = {}


def make_in_maps(x, Wq, Wk, Wv, Wp, bp):
    cosk, sink = _rope_tables()
    bf = ml_dtypes.bfloat16
    bpb = np.ascontiguousarray(np.tile(np.asarray(bp, np.float32)[None, :], (P, 1)))
    Wq = np.ascontiguousarray(np.asarray(Wq, np.float32).astype(bf))
    Wk = np.ascontiguousarray(np.asarray(Wk, np.float32).astype(bf))
    Wv = np.ascontiguousarray(np.asarray(Wv, np.float32).astype(bf))
    Wp = np.ascontiguousarray(np.asarray(Wp, np.float32).astype(bf))
    in_maps = []
    for c in range(8):
        b, qh = c // 2, c % 2
        xT = np.asarray(x[b], np.float32).T.astype(bf)  # [D, T]
        roll = qh * TQ
        in_maps.append(
            {
                "xT": np.ascontiguousarray(np.roll(xT, -roll, axis=1)),
                "Wq": Wq,
                "Wk": Wk,
                "Wv": Wv,
                "Wp": Wp,
                "bpb": bpb,
                "cosk": np.ascontiguousarray(np.roll(cosk, -roll, axis=1)),
                "sink": np.ascontiguousarray(np.roll(sink, -roll, axis=1)),
            }
        )
    return in_maps


def kernel(x, h, w, Wq, Wk, Wv, Wp, bp, _trace=False, **trace_kwargs):
    x = np.asarray(x, np.float32)
    in_maps = make_in_maps(x, Wq, Wk, Wv, Wp, bp)
    if "nc" not in _NC_CACHE:
        _NC_CACHE["nc"] = build_nc()
    nc = _NC_CACHE["nc"]
    res = run_bass_kernel_spmd(
        nc, in_maps, list(range(8)), trace=_trace, **trace_kwargs
    )
    out = np.empty((B, T, D), np.float32)
    for c in range(8):
        b, qh = c // 2, c % 2
        out[b, qh * TQ : (qh + 1) * TQ, :] = res.results[c]["out"]
    kernel.last_result = res
    return out


# revision 20
# speedup vs baseline: 1.0643x; 1.0643x over previous
"""AttentionWithRoPE on 8 Trainium2 NeuronCores.

Sharding: batch x query-half -> 8 independent cores (no collectives).
Core c handles batch b=c//2, query rows [qh*1024, (qh+1)*1024) with qh=c%2.
The host rolls the t axis per core so the query rows always sit in columns
[0, TQ) of xT; cosk/sink are rolled identically, so RoPE sees true positions
and the softmax key set is unchanged (order-invariant).

Per-core plan:
  Inputs x/Wq/Wk/Wv/Wp arrive bf16 (host-converted); V/attnout staging and
  the softmax P tiles are bf16; S = K^T.T@Q^T runs as an fp8e4 DoubleRow
  matmul (2x PE rate, head-dim packed [32,2] via SBUF repack DMAs); the
  final out projection accumulates in fp32 PSUM and is emitted fp32.
  V:    V projection in natural layout [t, j], staged to DRAM per-head
        contiguous (Vd3[h] is a ready-to-load [128, 16, 64] bf16 block; a
        ones column is appended in SBUF for softmax row sums).
  QK+attention, software-pipelined per head pair p:
        Q^T/K^T projections for pair p+1 (bf16, RoPE via signed pair-swap
        permutation matmul + DVE/GPSIMD combine, then cast-repacked to the
        fp8 [32,2,t] DoubleRow layout) are emitted interleaved with
        attention for pair p so the PE fills the gaps while ACT grinds
        through the softmax exps.
        Attention per head: S^T[t,l] via fp8 DoubleRow, P^T = exp(scale*S^T)
        on ACT in bf16 (logits are O(1), no max subtraction), attnout^T =
        [V_h|1].T @ P^T accumulated over t which also yields the softmax
        row sums in partition 64; normalize via reciprocal + GPSIMD
        partition-broadcast, then stage normalized attnout^T (bf16) to DRAM.
  Out:  out = attnout^T.T @ Wp + bp in fp32.
"""

import sys

sys.path.insert(0, "/opt/trn_rl_repo")

from contextlib import ExitStack

import ml_dtypes
import numpy as np

import concourse.bass as bass
import concourse.mybir as mybir
import concourse.tile as tile
from concourse.bass_utils import run_bass_kernel_spmd

F32 = mybir.dt.float32
F32R = mybir.dt.float32r
BF16 = mybir.dt.bfloat16
F8 = mybir.dt.float8e4
AF = mybir.ActivationFunctionType
MUL = mybir.AluOpType.mult
DR = mybir.MatmulPerfMode.DoubleRow

B, T, D = 4, 2048, 1024
H, HD = 16, 64
P = 128
TQ = 1024  # query rows per core
SCALE = float(D) ** -0.5
ROPE_THETA = 10000.0

_ws_ctr = [0]


def _split_multi_waits(nc):
    """The walrus build in this container accepts at most one sync-wait per
    engine instruction. Hoist all but one wait of each instruction into
    standalone EventSemaphore instructions on the same engine, placed
    immediately before it (engines are in-order, so semantics are identical)."""
    n = 0
    for f in nc.m.functions:
        for blk in f.blocks:
            insts = list(blk.instructions)
            newlist = []
            changed = False
            for inst in insts:
                si = getattr(inst, "sync_info", None)
                waits = list(si.on_wait) if si is not None and si.on_wait else []
                if len(waits) > 1:
                    for w in waits[:-1]:
                        _ws_ctr[0] += 1
                        evs = mybir.InstEventSemaphore(
                            name=f"WSPLIT-{_ws_ctr[0]}", ins=[], outs=[]
                        )
                        evs.engine = inst.engine
                        evs.sync_info = mybir.SyncInfo(on_wait=[w], on_update=[])
                        newlist.append(evs)
                        n += 1
                    inst.sync_info = mybir.SyncInfo(
                        on_wait=[waits[-1]], on_update=list(si.on_update)
                    )
                    changed = True
                newlist.append(inst)
            if changed:
                blk.instructions[:] = newlist
    return n


def _perm_lhsT():
    """lhsT for the rotate-half permutation: out = lhsT.T @ q gives
    out[2i] = -q[2i+1], out[2i+1] = q[2i]."""
    m = np.zeros((P, P), np.float32)
    for i in range(P // 2):
        m[2 * i + 1, 2 * i] = -1.0
        m[2 * i, 2 * i + 1] = 1.0
    return m


def build_nc(split_waits=True, reps=1):
    nc = bass.Bass(trn_type="TRN2", target_bir_lowering=False, debug=False)

    xT = nc.dram_tensor("xT", [D, T], BF16, kind="ExternalInput").ap()
    Wq = nc.dram_tensor("Wq", [D, D], BF16, kind="ExternalInput").ap()
    Wk = nc.dram_tensor("Wk", [D, D], BF16, kind="ExternalInput").ap()
    Wv = nc.dram_tensor("Wv", [D, D], BF16, kind="ExternalInput").ap()
    Wp = nc.dram_tensor("Wp", [D, D], BF16, kind="ExternalInput").ap()
    bpb = nc.dram_tensor("bpb", [P, D], F32, kind="ExternalInput").ap()
    cosk = nc.dram_tensor("cosk", [P, T], BF16, kind="ExternalInput").ap()
    sink = nc.dram_tensor("sink", [P, T], BF16, kind="ExternalInput").ap()
    out = nc.dram_tensor("out", [TQ, D], F32, kind="ExternalOutput").ap()

    permc = nc.inline_tensor(_perm_lhsT().astype(ml_dtypes.bfloat16), name="permc").ap()
    ones128c = nc.inline_tensor(np.ones((P, 64), np.float32), name="ones128c").ap()
    onescolc = nc.inline_tensor(
        np.ones((P, T // P, 1), ml_dtypes.bfloat16), name="onescolc"
    ).ap()

    # Vd3[h] is a contiguous [128(tp), 16(to), 64] bf16 per-head V block.
    # AOTd holds normalized attnout^T in bf16.
    Vd3 = nc.dram_tensor("Vd3", [H, P, T // P, HD], BF16).ap()
    AOTd = nc.dram_tensor("AOTd", [D // P, P, TQ], BF16).ap()

    xT_r = xT.rearrange("(do dp) t -> dp do t", dp=P)  # [128, 8, 2048]
    Wq_r = Wq.rearrange("(do dp) j -> dp do j", dp=P)
    Wk_r = Wk.rearrange("(do dp) j -> dp do j", dp=P)
    Wv_r = Wv.rearrange("(do dp) j -> dp do j", dp=P)
    Wp_r = Wp.rearrange("(ko kp) j -> kp ko j", kp=P)
    AOTd_r = AOTd.rearrange("ko p l -> p ko l")  # [128, 8, 1024]

    DO = D // P  # 8 contraction tiles
    NT = T // P  # 16 key tiles

    with tile.TileContext(nc) as tc:
      for _rep in range(reps):
        with ExitStack() as top:
            persist = top.enter_context(tc.tile_pool(name="persist", bufs=1))
            ones128 = persist.tile([P, 64], F32R)
            onesva = persist.tile([P, NT, 1], BF16)
            permt = persist.tile([P, P], BF16)
            ck = persist.tile([P, T], BF16)
            sk = persist.tile([P, T], BF16)
            vab = persist.tile([P, H, NT, 65], BF16)

            with tc.tile_pool(name="xpool", bufs=1) as xpool:
                xts = xpool.tile([P, DO, T], BF16)

                # -------- interleaved Q/K projection + attention pipeline ------
                with ExitStack() as pmain:
                    wv1pool = pmain.enter_context(tc.tile_pool(name="w_v1", bufs=1))
                    vpool = pmain.enter_context(tc.tile_pool(name="o_v1", bufs=2))
                    wpool = pmain.enter_context(tc.tile_pool(name="w_qk", bufs=2))
                    tpool = pmain.enter_context(tc.tile_pool(name="t_qk", bufs=2))
                    kqpool = pmain.enter_context(tc.tile_pool(name="kq", bufs=2))
                    k8pool = pmain.enter_context(tc.tile_pool(name="k8", bufs=2))
                    pmm = pmain.enter_context(
                        tc.tile_pool(name="ps_qk", bufs=1, space="PSUM")
                    )
                    pmisc = pmain.enter_context(
                        tc.tile_pool(name="ps_misc", bufs=1, space="PSUM")
                    )

                    def proj_half(ps, w_t, src_col, half):
                        """One contraction sweep over half the free dim."""
                        c0 = src_col * 512 + half * 256
                        for do in range(DO):
                            nc.tensor.matmul(
                                ps[:, half * 256 : half * 256 + 256],
                                w_t[:, do],
                                xts[:, do, c0 : c0 + 256],
                                start=(do == 0),
                                stop=(do == DO - 1),
                            )

                    def rope_tail(ps, dst, dst_col, src_col):
                        raw = tpool.tile([P, 512], BF16, tag="raw")
                        nc.vector.tensor_copy(raw[:], ps[:])
                        pr = pmisc.tile([P, 512], F32, tag="misc")
                        nc.tensor.matmul(pr[:], permt[:], raw[:], start=True, stop=True)
                        t1 = tpool.tile([P, 512], BF16, tag="t1")
                        nc.gpsimd.tensor_tensor(
                            t1[:],
                            raw[:],
                            ck[:, src_col * 512 : (src_col + 1) * 512],
                            MUL,
                        )
                        t2 = tpool.tile([P, 512], BF16, tag="t2")
                        nc.vector.tensor_mul(
                            t2[:], pr[:], sk[:, src_col * 512 : (src_col + 1) * 512]
                        )
                        nc.vector.tensor_add(
                            dst[:, dst_col * 512 : (dst_col + 1) * 512], t1[:], t2[:]
                        )

                    def emit_proj_jo(jo, preload=False, piecewise=False):
                        """Q^T/K^T for head pair jo as a list of small filler
                        closures (two matmul halves + rope tail per 512-col
                        tile, then fp8 [32,2] repack DMAs). Consecutive
                        closures sharing the pmm "ps" bank rely on FIFO
                        adjacency - nothing else allocates that tag between
                        pops."""
                        ktp = kqpool.tile([P, T], F8, tag="ktp")
                        qtp = kqpool.tile([P, TQ], F8, tag="qtp")
                        kp8 = k8pool.tile([P, 2, T], F8, tag="kp8")
                        qp8 = k8pool.tile([P, 2, TQ], F8, tag="qp8")
                        steps = []
                        wref = []
                        state = {}

                        def load_w():
                            wq_t = wpool.tile([P, DO, P], BF16, tag="wq")
                            nc.gpsimd.dma_start(
                                wq_t[:], Wq_r[:, :, jo * P : (jo + 1) * P]
                            )
                            wk_t = wpool.tile([P, DO, P], BF16, tag="wk")
                            nc.gpsimd.dma_start(
                                wk_t[:], Wk_r[:, :, jo * P : (jo + 1) * P]
                            )
                            return wq_t, wk_t

                        if preload:
                            wref.extend(load_w())

                        def half_a(which, col):
                            if not wref:
                                wref.extend(load_w())
                            w_t = wref[0] if which == "q" else wref[1]
                            ps = pmm.tile([P, 512], F32, tag="ps", name="psp")
                            state["ps"] = ps
                            proj_half(ps, w_t, col, 0)

                        def half_b(which, col):
                            w_t = wref[0] if which == "q" else wref[1]
                            ps = state["ps"]
                            proj_half(ps, w_t, col, 1)
                            dst = qtp if which == "q" else ktp
                            rope_tail(ps, dst, col, col)

                        def repack(src, dst, c0, c1):
                            for h in range(2):
                                for i in range(2):
                                    nc.sync.dma_start(
                                        dst[64 * h : 64 * h + 32, i, c0:c1],
                                        src[64 * h + 32 * i : 64 * h + 32 * i + 32,
                                            c0:c1],
                                    )

                        order = [("q", 0), ("k", 0), ("k", 1), ("k", 2), ("k", 3),
                                 ("q", 1)]
                        for which, col in order:
                            steps.append(lambda w=which, c=col: half_a(w, c))
                            steps.append(lambda w=which, c=col: half_b(w, c))
                            if piecewise:
                                if which == "q":
                                    steps.append(
                                        lambda c=col: repack(
                                            qtp, qp8, c * 512, (c + 1) * 512
                                        )
                                    )
                                else:
                                    steps.append(
                                        lambda c=col: repack(
                                            ktp, kp8, c * 512, (c + 1) * 512
                                        )
                                    )
                        if not piecewise:
                            steps.append(lambda: repack(ktp, kp8, 0, T))
                            steps.append(lambda: repack(qtp, qp8, 0, TQ))
                        return kp8, qp8, steps

                    # ------------- windowed attention pipeline ----------------
                    # 16 windows w = (pair p, query half lc). Window w emits
                    # S+exp for (p, lc) while retiring the PV accumulation and
                    # softmax normalization of window w-1, with projection /
                    # V-projection / output-projection steps as PE fillers.
                    vapool = pmain.enter_context(tc.tile_pool(name="va", bufs=2))
                    ptpool = pmain.enter_context(tc.tile_pool(name="pt", bufs=36))
                    smpool = pmain.enter_context(tc.tile_pool(name="sm", bufs=2))
                    vpool0 = pmain.enter_context(tc.tile_pool(name="o_v", bufs=2))
                    wvpool = pmain.enter_context(tc.tile_pool(name="w_v", bufs=1))
                    wppool = pmain.enter_context(tc.tile_pool(name="wp", bufs=1))
                    apool = pmain.enter_context(tc.tile_pool(name="aot", bufs=3))
                    outpool = pmain.enter_context(tc.tile_pool(name="outp", bufs=3))
                    pss = pmain.enter_context(
                        tc.tile_pool(name="ps_s", bufs=4, space="PSUM")
                    )
                    pso = pmain.enter_context(
                        tc.tile_pool(name="ps_o", bufs=1, space="PSUM")
                    )

                    kp80, qp80, steps0 = emit_proj_jo(0, preload=True, piecewise=True)
                    # V weights in three column bands: heads 0-1 first so the
                    # first PV sweep unblocks early, then heads 2-9, 10-15.
                    for q in range(2):
                        nc.sync.dma_start(
                            xts[:, :, q * 256 : (q + 1) * 256],
                            xT_r[:, :, q * 256 : (q + 1) * 256],
                        )
                    nc.gpsimd.dma_start(ck[:], cosk[:])
                    nc.gpsimd.dma_start(sk[:], sink[:])
                    nc.gpsimd.dma_start(permt[:], permc)
                    wv_a = wvpool.tile([P, DO, P], BF16, tag="wva")
                    nc.sync.dma_start(wv_a[:], Wv_r[:, :, 0:128])
                    for q in range(2, 8):
                        eng = (nc.sync, nc.gpsimd)[q % 2]
                        eng.dma_start(
                            xts[:, :, q * 256 : (q + 1) * 256],
                            xT_r[:, :, q * 256 : (q + 1) * 256],
                        )
                    nc.gpsimd.dma_start(ones128[:], ones128c.bitcast(F32R))
                    nc.gpsimd.dma_start(onesva[:], onescolc)
                    wv_b = wvpool.tile([P, DO, 512], BF16, tag="wvb")
                    wv_c = wvpool.tile([P, DO, 384], BF16, tag="wvc")
                    for hh in range(H):
                        nc.vector.tensor_copy(vab[:, hh, :, 64:65], onesva[:])

                    def v_band(band, to, half):
                        """V projection for one t-tile of a column band.
                        band 0: cols 0:128 (heads 0-1, single shot);
                        band 1: cols 128:640 in two halves;
                        band 2: cols 640:1024 in two halves."""
                        wv, j0, w = [
                            (wv_a, 0, 128), (wv_b, 128, 512), (wv_c, 640 - 128, 384)
                        ][band]
                        j0 = [0, 128, 640][band]
                        if band == 0:
                            cw, c0 = 128, 0
                        else:
                            cw, c0 = w // 2, half * (w // 2)
                        if half == 0 or band == 0:
                            ps = pmm.tile([P, 512], F32, tag="ps", name="psv")
                            state = ps
                            vps[to] = ps
                        ps = vps[to]
                        for do in range(DO):
                            nc.tensor.matmul(
                                ps[:, c0 : c0 + cw],
                                xts[:, do, to * P : (to + 1) * P],
                                wv[:, do, c0 : c0 + cw],
                                start=(do == 0),
                                stop=(do == DO - 1),
                            )
                        if half == 1 or band == 0:
                            h0, nh = j0 // 64, w // 64
                            nc.vector.tensor_copy(
                                vab[:, h0 : h0 + nh, to, 0:64],
                                ps[:, 0:w].rearrange("tp (h e) -> tp h e", e=64),
                            )

                    vps = {}

                    wp_ref = []

                    def load_wp():
                        wp_t = wppool.tile([P, DO, D], BF16, name="wp_t")
                        for ko in range(DO):
                            nc.sync.dma_start(wp_t[:, ko], Wp_r[:, ko])
                        bpt = wppool.tile([P, D], F32, name="bpt")
                        nc.sync.dma_start(bpt[:], bpb[:])
                        wp_ref.extend((wp_t, bpt))

                    aot_ref = {}

                    def out_load(lt):
                        aot_t = apool.tile([P, DO, P], BF16, tag="aot")
                        nc.sync.dma_start(
                            aot_t[:], AOTd_r[:, :, lt * P : (lt + 1) * P]
                        )
                        aot_ref[lt] = aot_t

                    def out_quarter(lt, jc, half):
                        wp_t, bpt = wp_ref
                        aot_t = aot_ref[lt]
                        j0 = jc * 512 + half * 256
                        if half == 0:
                            ps = pmm.tile([P, 512], F32, tag="ps", name="psf")
                            outps[lt] = ps
                        ps = outps[lt]
                        for ko in range(DO):
                            nc.tensor.matmul(
                                ps[:, half * 256 : half * 256 + 256],
                                aot_t[:, ko],
                                wp_t[:, ko, j0 : j0 + 256],
                                start=(ko == 0),
                                stop=(ko == DO - 1),
                            )
                        if half == 1:
                            ot = outpool.tile([P, 512], F32, tag="oto")
                            nc.vector.tensor_add(
                                ot[:], ps[:], bpt[:, jc * 512 : (jc + 1) * 512]
                            )
                            nc.sync.dma_start(
                                out[lt * P : (lt + 1) * P,
                                    jc * 512 : (jc + 1) * 512],
                                ot[:],
                            )

                    outps = {}

                    def out_steps(lt):
                        return [
                            lambda: out_load(lt),
                            lambda: out_quarter(lt, 0, 0),
                            lambda: out_quarter(lt, 0, 1),
                            lambda: out_quarter(lt, 1, 0),
                            lambda: out_quarter(lt, 1, 1),
                        ]

                    vvs_ref = {}

                    def make_pv_norm(w, pts_list):
                        p, lc = divmod(w, 2)
                        pos = [None, None]

                        def pv_step(to):
                            if to == 0:
                                pos[0] = pso.tile([P, 512], F32, tag="po0", name="po0")
                                pos[1] = pso.tile([P, 512], F32, tag="po1", name="po1")
                            for h in range(2):
                                nc.tensor.matmul(
                                    pos[h][0:65, :],
                                    vab[:, 2 * p + h, to, :],
                                    pts_list[to][h][:],
                                    start=(to == 0),
                                    stop=(to == NT - 1),
                                )

                        sm_ref = {}

                        def norm_a(h):
                            # DVE-only half: copy accumulator out of PSUM and
                            # take the row-sum reciprocal (frees pos fast).
                            po_s = smpool.tile([65, 512], F32, tag="pos")
                            nc.vector.tensor_copy(po_s[:], pos[h][0:65, :])
                            rc = smpool.tile([65, 512], F32R, tag="rc")
                            with nc.allow_low_precision(
                                reason="f32r feeds the broadcast matmul"
                            ):
                                nc.vector.reciprocal(rc[64:65, :], po_s[64:65, :])
                            sm_ref[h] = (po_s, rc)

                        def norm_b(h):
                            # PE-broadcast half, deferred into the next window.
                            po_s, rc = sm_ref[h]
                            pb = pmisc.tile(
                                [P, 512], F32, tag="misc", name="pb"
                            )[0:64, :]
                            nc.tensor.matmul(
                                pb[:],
                                ones128[64:65, :],
                                rc[64:65, :],
                                start=True,
                                stop=True,
                            )
                            rbb = smpool.tile([64, 512], F32, tag="rbb")
                            nc.vector.tensor_copy(rbb[:], pb[:])
                            tmpn = smpool.tile([64, 512], BF16, tag="tmpn")
                            nc.vector.tensor_mul(tmpn[:], po_s[0:64, :], rbb[:])
                            nc.sync.dma_start(
                                AOTd[p, 64 * h : 64 * h + 64,
                                     lc * 512 : (lc + 1) * 512],
                                tmpn[:],
                            )

                        return pv_step, norm_a, norm_b

                    fillers = []

                    def pop_fillers(k):
                        n = 0
                        while fillers and n < k:
                            fillers.pop(0)()
                            n += 1

                    def emit_window(w, kp8, qp8, prev, pop_k=1):
                        p, lc = divmod(w, 2)
                        pts_list = []
                        for to in range(NT):
                            pts = []
                            for h in range(2):
                                ps = pss.tile([P, 512], F32, tag="pss")
                                nc.tensor.matmul(
                                    ps[:],
                                    kp8[64 * h : 64 * h + 32, :,
                                        to * P : (to + 1) * P],
                                    qp8[64 * h : 64 * h + 32, :,
                                        lc * 512 : (lc + 1) * 512],
                                    start=True,
                                    stop=True,
                                    perf_mode=DR,
                                )
                                pt = ptpool.tile([P, 512], BF16, tag="pt")
                                nc.scalar.activation(pt[:], ps[:], AF.Exp, scale=SCALE)
                                pts.append(pt)
                            pts_list.append(pts)
                            if prev is not None:
                                prev[0](to)
                            pop_fillers(pop_k if len(fillers) < 12 else pop_k + 1)
                        if prev is not None:
                            prev[1](0)
                            prev[1](1)
                        nprev = make_pv_norm(w, pts_list)
                        if prev is not None:
                            fillers.insert(0, lambda: (prev[2](0), prev[2](1)))
                        return nprev

                    # jo0's first two tiles (q cols 0:512, k cols 0:512) must
                    # fully precede the first S consumer (in-order PE); the
                    # rest of jo0 flows through the filler FIFO ahead of the
                    # to-positions that consume each K tile.
                    for s in steps0[:6]:
                        s()
                    fillers.extend(steps0[6:])

                    prev = None
                    kp8c, qp8c = kp80, qp80
                    nkp8 = nqp8 = None
                    for w in range(16):
                        p, lc = divmod(w, 2)
                        if lc == 0 and p + 1 < DO:
                            nkp8, nqp8, nsteps = emit_proj_jo(p + 1)
                            fillers.extend(nsteps)
                        if w == 0:
                            fillers.extend(
                                (lambda to=to: v_band(0, to, 0)) for to in range(NT)
                            )
                        elif w in (1, 2):
                            if w == 1:
                                fillers.append(
                                    lambda: nc.sync.dma_start(
                                        wv_b[:], Wv_r[:, :, 128:640]
                                    )
                                )
                            rng = range(0, 10) if w == 1 else range(10, NT)
                            fillers.extend(
                                (lambda to=to, h=h: v_band(1, to, h))
                                for to in rng
                                for h in range(2)
                            )
                        elif w in (3, 4):
                            if w == 3:
                                fillers.append(
                                    lambda: nc.sync.dma_start(
                                        wv_c[:], Wv_r[:, :, 640:1024]
                                    )
                                )
                            rng = range(0, 10) if w == 3 else range(10, NT)
                            fillers.extend(
                                (lambda to=to, h=h: v_band(2, to, h))
                                for to in rng
                                for h in range(2)
                            )
                        if w == 12:
                            load_wp()
                        prev = emit_window(
                            w, kp8c, qp8c, prev, pop_k=(3 if w == 0 else 1)
                        )
                        if lc == 1 and nkp8 is not None:
                            kp8c, qp8c = nkp8, nqp8
                            nkp8 = nqp8 = None

                    # drain: final window's PV + norms, then output projection
                    pv_f, norm_a_f, norm_b_f = prev
                    for lt in range(4):
                        fillers.extend(out_steps(lt))
                    for to in range(NT):
                        pv_f(to)
                        pop_fillers(2)
                    norm_a_f(0)
                    norm_a_f(1)
                    pop_fillers(len(fillers))
                    norm_b_f(0)
                    norm_b_f(1)
                    for lt in range(4, TQ // P):
                        for s in out_steps(lt):
                            s()

    if split_waits:
        _split_multi_waits(nc)
    return nc


def _rope_tables():
    inv = 1.0 / (ROPE_THETA ** (np.arange(0, HD, 2, dtype=np.float32) / HD))
    t = np.arange(T, dtype=np.float32)
    freqs = np.einsum("i,j->ij", t, inv)  # [T, 32]
    freqs = np.repeat(freqs, 2, axis=-1)  # [T, 64]
    cosT = np.cos(freqs).T  # [64, T]
    sinT = np.sin(freqs).T
    cosk = np.tile(cosT, (2, 1)).astype(np.float32)  # [128, T]
    sink = np.tile(sinT, (2, 1)).astype(np.float32)
    return np.ascontiguousarray(cosk), np.ascontiguousarray(sink)


_NC_CACHE = {}


def make_in_maps(x, Wq, Wk, Wv, Wp, bp):
    cosk, sink = _rope_tables()
    bf = ml_dtypes.bfloat16
    bpb = np.ascontiguousarray(np.tile(np.asarray(bp, np.float32)[None, :], (P, 1)))
    Wq = np.ascontiguousarray(np.asarray(Wq, np.float32).astype(bf))
    Wk = np.ascontiguousarray(np.asarray(Wk, np.float32).astype(bf))
    Wv = np.ascontiguousarray(np.asarray(Wv, np.float32).astype(bf))
    Wp = np.ascontiguousarray(np.asarray(Wp, np.float32).astype(bf))
    in_maps = []
    for c in range(8):
        b, qh = c // 2, c % 2
        xT = np.asarray(x[b], np.float32).T.astype(bf)  # [D, T]
        roll = qh * TQ
        in_maps.append(
            {
                "xT": np.ascontiguousarray(np.roll(xT, -roll, axis=1)),
                "Wq": Wq,
                "Wk": Wk,
                "Wv": Wv,
                "Wp": Wp,
                "bpb": bpb,
                "cosk": np.ascontiguousarray(np.roll(cosk, -roll, axis=1)).astype(bf),
                "sink": np.ascontiguousarray(np.roll(sink, -roll, axis=1)).astype(bf),
            }
        )
    return in_maps


def kernel(x, h, w, Wq, Wk, Wv, Wp, bp, _trace=False, **trace_kwargs):
    x = np.asarray(x, np.float32)
    in_maps = make_in_maps(x, Wq, Wk, Wv, Wp, bp)
    if "nc" not in _NC_CACHE:
        _NC_CACHE["nc"] = build_nc()
    nc = _NC_CACHE["nc"]
    res = run_bass_kernel_spmd(
        nc, in_maps, list(range(8)), trace=_trace, **trace_kwargs
    )
    out = np.empty((B, T, D), np.float32)
    for c in range(8):
        b, qh = c // 2, c % 2
        out[b, qh * TQ : (qh + 1) * TQ, :] = res.results[c]["out"]
    kernel.last_result = res
    return out


# revision 28
# speedup vs baseline: 1.0942x; 1.0281x over previous
"""AttentionWithRoPE on 8 Trainium2 NeuronCores.

Sharding: batch x query-half -> 8 independent cores (no collectives).
Core c handles batch b=c//2, query rows [qh*1024, (qh+1)*1024) with qh=c%2.
The host rolls the t axis per core so the query rows always sit in columns
[0, TQ) of xT; cosk/sink are rolled identically, so RoPE sees true positions
and the softmax key set is unchanged (order-invariant).

Per-core plan:
  Inputs x/Wq/Wk/Wv/Wp arrive bf16 (host-converted); V/attnout staging and
  the softmax P tiles are bf16; S = K^T.T@Q^T runs as an fp8e4 DoubleRow
  matmul (2x PE rate, head-dim packed [32,2] via SBUF repack DMAs); the
  final out projection accumulates in fp32 PSUM and is emitted fp32.
  V:    V projection in natural layout [t, j], staged to DRAM per-head
        contiguous (Vd3[h] is a ready-to-load [128, 16, 64] bf16 block; a
        ones column is appended in SBUF for softmax row sums).
  QK+attention, software-pipelined per head pair p:
        Q^T/K^T projections for pair p+1 (bf16, RoPE via signed pair-swap
        permutation matmul + DVE/GPSIMD combine, then cast-repacked to the
        fp8 [32,2,t] DoubleRow layout) are emitted interleaved with
        attention for pair p so the PE fills the gaps while ACT grinds
        through the softmax exps.
        Attention per head: S^T[t,l] via fp8 DoubleRow, P^T = exp(scale*S^T)
        on ACT in bf16 (logits are O(1), no max subtraction), attnout^T =
        [V_h|1].T @ P^T accumulated over t which also yields the softmax
        row sums in partition 64; normalize via reciprocal + GPSIMD
        partition-broadcast, then stage normalized attnout^T (bf16) to DRAM.
  Out:  out = attnout^T.T @ Wp + bp in fp32.
"""

import sys

sys.path.insert(0, "/opt/trn_rl_repo")

from contextlib import ExitStack

import ml_dtypes
import numpy as np

import concourse.bass as bass
import concourse.mybir as mybir
import concourse.tile as tile
from concourse.bass_utils import run_bass_kernel_spmd

F32 = mybir.dt.float32
F32R = mybir.dt.float32r
BF16 = mybir.dt.bfloat16
F8 = mybir.dt.float8e4
AF = mybir.ActivationFunctionType
MUL = mybir.AluOpType.mult
DR = mybir.MatmulPerfMode.DoubleRow

B, T, D = 4, 2048, 1024
H, HD = 16, 64
P = 128
TQ = 1024  # query rows per core
SCALE = float(D) ** -0.5
ROPE_THETA = 10000.0

_ws_ctr = [0]


def _split_multi_waits(nc):
    """The walrus build in this container accepts at most one sync-wait per
    engine instruction. Hoist all but one wait of each instruction into
    standalone EventSemaphore instructions on the same engine, placed
    immediately before it (engines are in-order, so semantics are identical)."""
    n = 0
    for f in nc.m.functions:
        for blk in f.blocks:
            insts = list(blk.instructions)
            newlist = []
            changed = False
            for inst in insts:
                si = getattr(inst, "sync_info", None)
                waits = list(si.on_wait) if si is not None and si.on_wait else []
                if len(waits) > 1:
                    for w in waits[:-1]:
                        _ws_ctr[0] += 1
                        evs = mybir.InstEventSemaphore(
                            name=f"WSPLIT-{_ws_ctr[0]}", ins=[], outs=[]
                        )
                        evs.engine = inst.engine
                        evs.sync_info = mybir.SyncInfo(on_wait=[w], on_update=[])
                        newlist.append(evs)
                        n += 1
                    inst.sync_info = mybir.SyncInfo(
                        on_wait=[waits[-1]], on_update=list(si.on_update)
                    )
                    changed = True
                newlist.append(inst)
            if changed:
                blk.instructions[:] = newlist
    return n


def _perm_lhsT():
    """lhsT for the rotate-half permutation: out = lhsT.T @ q gives
    out[2i] = -q[2i+1], out[2i+1] = q[2i]."""
    m = np.zeros((P, P), np.float32)
    for i in range(P // 2):
        m[2 * i + 1, 2 * i] = -1.0
        m[2 * i, 2 * i + 1] = 1.0
    return m


def build_nc(split_waits=True, reps=1):
    nc = bass.Bass(trn_type="TRN2", target_bir_lowering=False, debug=False)

    xT = nc.dram_tensor("xT", [D, T], BF16, kind="ExternalInput").ap()
    Wq = nc.dram_tensor("Wq", [D, D], BF16, kind="ExternalInput").ap()
    Wk = nc.dram_tensor("Wk", [D, D], BF16, kind="ExternalInput").ap()
    Wv = nc.dram_tensor("Wv", [D, D], BF16, kind="ExternalInput").ap()
    Wp = nc.dram_tensor("Wp", [D, D], BF16, kind="ExternalInput").ap()
    bpb = nc.dram_tensor("bpb", [P, D], BF16, kind="ExternalInput").ap()
    cosk = nc.dram_tensor("cosk", [P, T], BF16, kind="ExternalInput").ap()
    sink = nc.dram_tensor("sink", [P, T], BF16, kind="ExternalInput").ap()
    out = nc.dram_tensor("out", [TQ, D], F32, kind="ExternalOutput").ap()

    permc = nc.inline_tensor(_perm_lhsT().astype(ml_dtypes.bfloat16), name="permc").ap()
    ones128c = nc.inline_tensor(np.ones((P, 64), np.float32), name="ones128c").ap()
    onescolc = nc.inline_tensor(
        np.ones((P, T // P, 1), ml_dtypes.bfloat16), name="onescolc"
    ).ap()

    # Vd3[h] is a contiguous [128(tp), 16(to), 64] bf16 per-head V block.
    # AOTd holds normalized attnout^T in bf16.
    Vd3 = nc.dram_tensor("Vd3", [H, P, T // P, HD], BF16).ap()
    AOTd = nc.dram_tensor("AOTd", [D // P, P, TQ], BF16).ap()

    xT_r = xT.rearrange("(do dp) t -> dp do t", dp=P)  # [128, 8, 2048]
    Wq_r = Wq.rearrange("(do dp) j -> dp do j", dp=P)
    Wk_r = Wk.rearrange("(do dp) j -> dp do j", dp=P)
    Wv_r = Wv.rearrange("(do dp) j -> dp do j", dp=P)
    Wp_r = Wp.rearrange("(ko kp) j -> kp ko j", kp=P)
    AOTd_r = AOTd.rearrange("ko p l -> p ko l")  # [128, 8, 1024]

    DO = D // P  # 8 contraction tiles
    NT = T // P  # 16 key tiles

    with tile.TileContext(nc) as tc:
      for _rep in range(reps):
        with ExitStack() as top:
            persist = top.enter_context(tc.tile_pool(name="persist", bufs=1))
            ones128 = persist.tile([P, 64], F32R)
            onesva = persist.tile([P, NT, 1], BF16)
            permt = persist.tile([P, P], BF16)
            ck = persist.tile([P, T], BF16)
            sk = persist.tile([P, T], BF16)
            vab = persist.tile([P, H, NT, 65], BF16)

            with tc.tile_pool(name="xpool", bufs=1) as xpool:
                xts = xpool.tile([P, DO, T], BF16)

                # -------- interleaved Q/K projection + attention pipeline ------
                with ExitStack() as pmain:
                    wv1pool = pmain.enter_context(tc.tile_pool(name="w_v1", bufs=1))
                    vpool = pmain.enter_context(tc.tile_pool(name="o_v1", bufs=2))
                    wpool = pmain.enter_context(tc.tile_pool(name="w_qk", bufs=2))
                    tpool = pmain.enter_context(tc.tile_pool(name="t_qk", bufs=2))
                    kqpool = pmain.enter_context(tc.tile_pool(name="kq", bufs=2))
                    k8pool = pmain.enter_context(tc.tile_pool(name="k8", bufs=2))
                    pmm = pmain.enter_context(
                        tc.tile_pool(name="ps_qk", bufs=1, space="PSUM")
                    )
                    pmisc = pmain.enter_context(
                        tc.tile_pool(name="ps_misc", bufs=1, space="PSUM")
                    )

                    def proj_half(ps, w_t, src_col, half):
                        """One contraction sweep over half the free dim."""
                        c0 = src_col * 512 + half * 256
                        for do in range(DO):
                            nc.tensor.matmul(
                                ps[:, half * 256 : half * 256 + 256],
                                w_t[:, do],
                                xts[:, do, c0 : c0 + 256],
                                start=(do == 0),
                                stop=(do == DO - 1),
                            )

                    def rope_tail(ps, dst, dst_col, src_col):
                        raw = tpool.tile([P, 512], BF16, tag="raw")
                        nc.vector.tensor_copy(raw[:], ps[:])
                        pr = pmisc.tile([P, 512], F32, tag="misc")
                        nc.tensor.matmul(pr[:], permt[:], raw[:], start=True, stop=True)
                        t1 = tpool.tile([P, 512], BF16, tag="t1")
                        nc.gpsimd.tensor_tensor(
                            t1[:],
                            raw[:],
                            ck[:, src_col * 512 : (src_col + 1) * 512],
                            MUL,
                        )
                        t2 = tpool.tile([P, 512], BF16, tag="t2")
                        nc.vector.tensor_mul(
                            t2[:], pr[:], sk[:, src_col * 512 : (src_col + 1) * 512]
                        )
                        nc.vector.tensor_add(
                            dst[:, dst_col * 512 : (dst_col + 1) * 512], t1[:], t2[:]
                        )

                    def emit_proj_jo(jo, preload=False, piecewise=False):
                        """Q^T/K^T for head pair jo as a list of small filler
                        closures (two matmul halves + rope tail per 512-col
                        tile, then fp8 [32,2] repack DMAs). Consecutive
                        closures sharing the pmm "ps" bank rely on FIFO
                        adjacency - nothing else allocates that tag between
                        pops."""
                        ktp = kqpool.tile([P, T], F8, tag="ktp")
                        qtp = kqpool.tile([P, TQ], F8, tag="qtp")
                        kp8 = k8pool.tile([P, 2, T], F8, tag="kp8")
                        qp8 = k8pool.tile([P, 2, TQ], F8, tag="qp8")
                        steps = []
                        wref = []
                        state = {}

                        def load_w():
                            wq_t = wpool.tile([P, DO, P], BF16, tag="wq")
                            nc.gpsimd.dma_start(
                                wq_t[:], Wq_r[:, :, jo * P : (jo + 1) * P]
                            )
                            wk_t = wpool.tile([P, DO, P], BF16, tag="wk")
                            nc.gpsimd.dma_start(
                                wk_t[:], Wk_r[:, :, jo * P : (jo + 1) * P]
                            )
                            return wq_t, wk_t

                        if preload:
                            wref.extend(load_w())

                        def half_a(which, col):
                            if not wref:
                                wref.extend(load_w())
                            w_t = wref[0] if which == "q" else wref[1]
                            ps = pmm.tile([P, 512], F32, tag="ps", name="psp")
                            state["ps"] = ps
                            proj_half(ps, w_t, col, 0)

                        def half_b(which, col):
                            w_t = wref[0] if which == "q" else wref[1]
                            ps = state["ps"]
                            proj_half(ps, w_t, col, 1)
                            dst = qtp if which == "q" else ktp
                            rope_tail(ps, dst, col, col)

                        def repack(src, dst, c0, c1):
                            for h in range(2):
                                for i in range(2):
                                    nc.sync.dma_start(
                                        dst[64 * h : 64 * h + 32, i, c0:c1],
                                        src[64 * h + 32 * i : 64 * h + 32 * i + 32,
                                            c0:c1],
                                    )

                        order = [("q", 0), ("k", 0), ("k", 1), ("k", 2), ("k", 3),
                                 ("q", 1)]
                        for which, col in order:
                            steps.append(lambda w=which, c=col: half_a(w, c))
                            steps.append(lambda w=which, c=col: half_b(w, c))
                            if piecewise:
                                if which == "q":
                                    steps.append(
                                        lambda c=col: repack(
                                            qtp, qp8, c * 512, (c + 1) * 512
                                        )
                                    )
                                else:
                                    steps.append(
                                        lambda c=col: repack(
                                            ktp, kp8, c * 512, (c + 1) * 512
                                        )
                                    )
                        if not piecewise:
                            steps.append(lambda: repack(ktp, kp8, 0, T))
                            steps.append(lambda: repack(qtp, qp8, 0, TQ))
                        return kp8, qp8, steps

                    # ------------- windowed attention pipeline ----------------
                    # 16 windows w = (pair p, query half lc). Window w emits
                    # S+exp for (p, lc) while retiring the PV accumulation and
                    # softmax normalization of window w-1, with projection /
                    # V-projection / output-projection steps as PE fillers.
                    vapool = pmain.enter_context(tc.tile_pool(name="va", bufs=2))
                    ptpool = pmain.enter_context(tc.tile_pool(name="pt", bufs=36))
                    smpool = pmain.enter_context(tc.tile_pool(name="sm", bufs=2))
                    vpool0 = pmain.enter_context(tc.tile_pool(name="o_v", bufs=2))
                    wvpool = pmain.enter_context(tc.tile_pool(name="w_v", bufs=1))
                    wppool = pmain.enter_context(tc.tile_pool(name="wp", bufs=1))
                    apool = pmain.enter_context(tc.tile_pool(name="aot", bufs=3))
                    outpool = pmain.enter_context(tc.tile_pool(name="outp", bufs=2))
                    pss = pmain.enter_context(
                        tc.tile_pool(name="ps_s", bufs=4, space="PSUM")
                    )
                    pso = pmain.enter_context(
                        tc.tile_pool(name="ps_o", bufs=1, space="PSUM")
                    )

                    kp80, qp80, steps0 = emit_proj_jo(0, preload=True, piecewise=True)
                    # V weights in three column bands: heads 0-1 first so the
                    # first PV sweep unblocks early, then heads 2-9, 10-15.
                    for q in range(2):
                        nc.sync.dma_start(
                            xts[:, :, q * 256 : (q + 1) * 256],
                            xT_r[:, :, q * 256 : (q + 1) * 256],
                        )
                    nc.sync.dma_start(ck[:], cosk[:])
                    nc.sync.dma_start(sk[:], sink[:])
                    nc.sync.dma_start(permt[:], permc)
                    wv_a = wvpool.tile([P, DO, P], BF16, tag="wva")
                    nc.sync.dma_start(wv_a[:], Wv_r[:, :, 0:128])
                    for q in range(2, 8):
                        nc.sync.dma_start(
                            xts[:, :, q * 256 : (q + 1) * 256],
                            xT_r[:, :, q * 256 : (q + 1) * 256],
                        )
                    nc.sync.dma_start(ones128[:], ones128c.bitcast(F32R))
                    nc.sync.dma_start(onesva[:], onescolc)
                    wv_b = wvpool.tile([P, DO, 512], BF16, tag="bigw", name="wv_b")
                    wv_c = wvpool.tile([P, DO, 384], BF16, tag="bigw", name="wv_c")
                    for hh in range(H):
                        nc.vector.tensor_copy(vab[:, hh, :, 64:65], onesva[:])

                    def v_band(band, to, half):
                        """V projection for one t-tile of a column band.
                        band 0: cols 0:128 (heads 0-1, single shot);
                        band 1: cols 128:640 in two halves;
                        band 2: cols 640:1024 in two halves."""
                        wv, j0, w = [
                            (wv_a, 0, 128), (wv_b, 128, 512), (wv_c, 640 - 128, 384)
                        ][band]
                        j0 = [0, 128, 640][band]
                        if band == 0:
                            cw, c0 = 128, 0
                        else:
                            cw, c0 = w // 2, half * (w // 2)
                        if half == 0 or band == 0:
                            ps = pmm.tile([P, 512], F32, tag="ps", name="psv")
                            state = ps
                            vps[to] = ps
                        ps = vps[to]
                        for do in range(DO):
                            nc.tensor.matmul(
                                ps[:, c0 : c0 + cw],
                                xts[:, do, to * P : (to + 1) * P],
                                wv[:, do, c0 : c0 + cw],
                                start=(do == 0),
                                stop=(do == DO - 1),
                            )
                        if half == 1 or band == 0:
                            h0, nh = j0 // 64, w // 64
                            nc.vector.tensor_copy(
                                vab[:, h0 : h0 + nh, to, 0:64],
                                ps[:, 0:w].rearrange("tp (h e) -> tp h e", e=64),
                            )

                    vps = {}

                    wp_ref = []

                    def load_wp():
                        wp_t = wvpool.tile([P, DO, D], BF16, tag="bigw", name="wp_t")
                        for ko in range(DO):
                            nc.sync.dma_start(wp_t[:, ko], Wp_r[:, ko])
                        bpt = wppool.tile([P, D], BF16, name="bpt")
                        nc.sync.dma_start(bpt[:], bpb[:])
                        wp_ref.extend((wp_t, bpt))

                    o1pool = pmain.enter_context(tc.tile_pool(name="o1", bufs=8))
                    aot_ref = {}
                    o1_ref = {}
                    outps = {}

                    def out_load(lt, ph):
                        aot_t = apool.tile([P, 4, P], BF16, tag="aot")
                        nc.sync.dma_start(
                            aot_t[:],
                            AOTd_r[:, 4 * ph : 4 * ph + 4, lt * P : (lt + 1) * P],
                        )
                        aot_ref[lt, ph] = aot_t
                        if ph == 0:
                            o1_ref[lt] = o1pool.tile([P, TQ], BF16, tag="o1", name="o1t")

                    def out_quarter(lt, ph, jc, half):
                        wp_t, bpt = wp_ref
                        aot_t = aot_ref[lt, ph]
                        j0 = jc * 512 + half * 256
                        if half == 0:
                            ps = pmm.tile([P, 512], F32, tag="ps", name="psf")
                            outps[lt] = ps
                        ps = outps[lt]
                        for ko in range(4):
                            nc.tensor.matmul(
                                ps[:, half * 256 : half * 256 + 256],
                                aot_t[:, ko],
                                wp_t[:, 4 * ph + ko, j0 : j0 + 256],
                                start=(ko == 0),
                                stop=(ko == 3),
                            )
                        if half == 1:
                            o1s = o1_ref[lt][:, jc * 512 : (jc + 1) * 512]
                            if ph == 0:
                                nc.vector.tensor_copy(o1s, ps[:])
                            else:
                                ot = outpool.tile([P, 512], F32, tag="oto")
                                nc.vector.tensor_add(
                                    ot[:], ps[:], bpt[:, jc * 512 : (jc + 1) * 512]
                                )
                                ot2 = outpool.tile([P, 512], F32, tag="oto2")
                                nc.vector.tensor_add(ot2[:], ot[:], o1s)
                                nc.sync.dma_start(
                                    out[lt * P : (lt + 1) * P,
                                        jc * 512 : (jc + 1) * 512],
                                    ot2[:],
                                )

                    def out_steps(lt, ph):
                        return [
                            lambda: out_load(lt, ph),
                            lambda: out_quarter(lt, ph, 0, 0),
                            lambda: out_quarter(lt, ph, 0, 1),
                            lambda: out_quarter(lt, ph, 1, 0),
                            lambda: out_quarter(lt, ph, 1, 1),
                        ]

                    vvs_ref = {}

                    def make_pv_norm(w, pts_list):
                        p, lc = divmod(w, 2)
                        pos = [None, None]

                        def pv_step(to):
                            if to == 0:
                                pos[0] = pso.tile([P, 512], F32, tag="po0", name="po0")
                                pos[1] = pso.tile([P, 512], F32, tag="po1", name="po1")
                            for h in range(2):
                                nc.tensor.matmul(
                                    pos[h][0:65, :],
                                    vab[:, 2 * p + h, to, :],
                                    pts_list[to][h][:],
                                    start=(to == 0),
                                    stop=(to == NT - 1),
                                )

                        sm_ref = {}

                        def norm_a(h):
                            # DVE-only half: copy accumulator out of PSUM and
                            # take the row-sum reciprocal (frees pos fast).
                            po_s = smpool.tile([65, 512], F32, tag="pos")
                            nc.vector.tensor_copy(po_s[:], pos[h][0:65, :])
                            rc = smpool.tile([65, 512], F32R, tag="rc")
                            with nc.allow_low_precision(
                                reason="f32r feeds the broadcast matmul"
                            ):
                                nc.vector.reciprocal(rc[64:65, :], po_s[64:65, :])
                            sm_ref[h] = (po_s, rc)

                        def norm_b(h):
                            # PE-broadcast half, deferred into the next window.
                            po_s, rc = sm_ref[h]
                            pb = pmisc.tile(
                                [P, 512], F32, tag="misc", name="pb"
                            )[0:64, :]
                            nc.tensor.matmul(
                                pb[:],
                                ones128[64:65, :],
                                rc[64:65, :],
                                start=True,
                                stop=True,
                            )
                            rbb = smpool.tile([64, 512], F32, tag="rbb")
                            nc.vector.tensor_copy(rbb[:], pb[:])
                            tmpn = smpool.tile([64, 512], BF16, tag="tmpn")
                            nc.vector.tensor_mul(tmpn[:], po_s[0:64, :], rbb[:])
                            nc.sync.dma_start(
                                AOTd[p, 64 * h : 64 * h + 64,
                                     lc * 512 : (lc + 1) * 512],
                                tmpn[:],
                            )

                        return pv_step, norm_a, norm_b

                    fillers = []

                    def pop_fillers(k):
                        n = 0
                        while fillers and n < k:
                            fillers.pop(0)()
                            n += 1

                    def emit_window(w, kp8, qp8, prev, pop_k=1):
                        p, lc = divmod(w, 2)
                        pts_list = []
                        for to in range(NT):
                            pts = []
                            for h in range(2):
                                ps = pss.tile([P, 512], F32, tag="pss")
                                nc.tensor.matmul(
                                    ps[:],
                                    kp8[64 * h : 64 * h + 32, :,
                                        to * P : (to + 1) * P],
                                    qp8[64 * h : 64 * h + 32, :,
                                        lc * 512 : (lc + 1) * 512],
                                    start=True,
                                    stop=True,
                                    perf_mode=DR,
                                )
                                pt = ptpool.tile([P, 512], BF16, tag="pt")
                                nc.scalar.activation(pt[:], ps[:], AF.Exp, scale=SCALE)
                                pts.append(pt)
                            pts_list.append(pts)
                            if prev is not None:
                                prev[0](to)
                            pop_fillers(pop_k if len(fillers) < 12 else pop_k + 1)
                        if prev is not None:
                            prev[1](0)
                            prev[1](1)
                        nprev = make_pv_norm(w, pts_list)
                        if prev is not None:
                            fillers.insert(0, lambda: (prev[2](0), prev[2](1)))
                        return nprev

                    # jo0's first two tiles (q cols 0:512, k cols 0:512) must
                    # fully precede the first S consumer (in-order PE); the
                    # rest of jo0 flows through the filler FIFO ahead of the
                    # to-positions that consume each K tile.
                    for s in steps0[:6]:
                        s()
                    fillers.extend(steps0[6:])

                    prev = None
                    kp8c, qp8c = kp80, qp80
                    nkp8 = nqp8 = None
                    for w in range(16):
                        p, lc = divmod(w, 2)
                        if lc == 0 and p + 1 < DO:
                            nkp8, nqp8, nsteps = emit_proj_jo(p + 1)
                            fillers.extend(nsteps)
                        if w == 0:
                            fillers.extend(
                                (lambda to=to: v_band(0, to, 0)) for to in range(NT)
                            )
                        elif w in (1, 2):
                            if w == 1:
                                fillers.append(
                                    lambda: nc.sync.dma_start(
                                        wv_b[:], Wv_r[:, :, 128:640]
                                    )
                                )
                            rng = range(0, 10) if w == 1 else range(10, NT)
                            fillers.extend(
                                (lambda to=to, h=h: v_band(1, to, h))
                                for to in rng
                                for h in range(2)
                            )
                        elif w in (3, 4):
                            if w == 3:
                                fillers.append(
                                    lambda: nc.sync.dma_start(
                                        wv_c[:], Wv_r[:, :, 640:1024]
                                    )
                                )
                            rng = range(0, 10) if w == 3 else range(10, NT)
                            fillers.extend(
                                (lambda to=to, h=h: v_band(2, to, h))
                                for to in rng
                                for h in range(2)
                            )
                        if w == 9:
                            load_wp()
                        if w == 10:
                            for lt in range(4):
                                fillers.extend(out_steps(lt, 0))
                        if w == 12:
                            for lt in range(4, TQ // P):
                                fillers.extend(out_steps(lt, 0))
                        prev = emit_window(
                            w, kp8c, qp8c, prev, pop_k=(3 if w == 0 else 1)
                        )
                        if lc == 1 and nkp8 is not None:
                            kp8c, qp8c = nkp8, nqp8
                            nkp8 = nqp8 = None

                    # drain: final window's PV + norms, then output projection
                    pv_f, norm_a_f, norm_b_f = prev
                    for lt in range(4):
                        fillers.extend(out_steps(lt, 1))
                    for to in range(NT):
                        pv_f(to)
                        pop_fillers(3)
                    norm_a_f(0)
                    norm_a_f(1)
                    pop_fillers(len(fillers))
                    norm_b_f(0)
                    norm_b_f(1)
                    for lt in range(4, TQ // P):
                        out_load(lt, 1)
                    for lt in range(4, TQ // P):
                        for s in out_steps(lt, 1)[1:]:
                            s()
                    pop_fillers(len(fillers))

    if split_waits:
        _split_multi_waits(nc)
    return nc


def _rope_tables():
    inv = 1.0 / (ROPE_THETA ** (np.arange(0, HD, 2, dtype=np.float32) / HD))
    t = np.arange(T, dtype=np.float32)
    freqs = np.einsum("i,j->ij", t, inv)  # [T, 32]
    freqs = np.repeat(freqs, 2, axis=-1)  # [T, 64]
    cosT = np.cos(freqs).T  # [64, T]
    sinT = np.sin(freqs).T
    cosk = np.tile(cosT, (2, 1)).astype(np.float32)  # [128, T]
    sink = np.tile(sinT, (2, 1)).astype(np.float32)
    return np.ascontiguousarray(cosk), np.ascontiguousarray(sink)


_NC_CACHE = {}


def make_in_maps(x, Wq, Wk, Wv, Wp, bp):
    cosk, sink = _rope_tables()
    bf = ml_dtypes.bfloat16
    bpb = np.ascontiguousarray(np.tile(np.asarray(bp, np.float32)[None, :].astype(bf), (P, 1)))
    Wq = np.ascontiguousarray(np.asarray(Wq, np.float32).astype(bf))
    Wk = np.ascontiguousarray(np.asarray(Wk, np.float32).astype(bf))
    Wv = np.ascontiguousarray(np.asarray(Wv, np.float32).astype(bf))
    Wp = np.ascontiguousarray(np.asarray(Wp, np.float32).astype(bf))
    in_maps = []
    for c in range(8):
        b, qh = c // 2, c % 2
        xT = np.asarray(x[b], np.float32).T.astype(bf)  # [D, T]
        roll = qh * TQ
        in_maps.append(
            {
                "xT": np.ascontiguousarray(np.roll(xT, -roll, axis=1)),
                "Wq": Wq,
                "Wk": Wk,
                "Wv": Wv,
                "Wp": Wp,
                "bpb": bpb,
                "cosk": np.ascontiguousarray(np.roll(cosk, -roll, axis=1)).astype(bf),
                "sink": np.ascontiguousarray(np.roll(sink, -roll, axis=1)).astype(bf),
            }
        )
    return in_maps


def kernel(x, h, w, Wq, Wk, Wv, Wp, bp, _trace=False, **trace_kwargs):
    x = np.asarray(x, np.float32)
    in_maps = make_in_maps(x, Wq, Wk, Wv, Wp, bp)
    if "nc" not in _NC_CACHE:
        _NC_CACHE["nc"] = build_nc()
    nc = _NC_CACHE["nc"]
    res = run_bass_kernel_spmd(
        nc, in_maps, list(range(8)), trace=_trace, **trace_kwargs
    )
    out = np.empty((B, T, D), np.float32)
    for c in range(8):
        b, qh = c // 2, c % 2
        out[b, qh * TQ : (qh + 1) * TQ, :] = res.results[c]["out"]
    kernel.last_result = res
    return out


# revision 31
# speedup vs baseline: 1.1027x; 1.0077x over previous
"""AttentionWithRoPE on 8 Trainium2 NeuronCores.

Sharding: batch x query-half -> 8 independent cores (no collectives).
Core c handles batch b=c//2, query rows [qh*1024, (qh+1)*1024) with qh=c%2.
The host rolls the t axis per core so the query rows always sit in columns
[0, TQ) of xT; cosk/sink are rolled identically, so RoPE sees true positions
and the softmax key set is unchanged (order-invariant).

Per-core plan:
  Inputs x/Wq/Wk/Wv/Wp arrive bf16 (host-converted); V/attnout staging and
  the softmax P tiles are bf16; S = K^T.T@Q^T runs as an fp8e4 DoubleRow
  matmul (2x PE rate, head-dim packed [32,2] via SBUF repack DMAs); the
  final out projection accumulates in fp32 PSUM and is emitted fp32.
  V:    V projection in natural layout [t, j], reshuffled per-head into a
        resident SBUF block vab[tp, h, to, 65] by DVE copies (no DRAM
        roundtrip); column 64 holds ones for the softmax row sums.
  Attention runs as 16 windows w = (head pair p, query half lc): window w
        emits S+exp for (p, lc) while retiring window w-1's PV accumulation
        and softmax normalization, with the next pair's Q^T/K^T projection
        (bf16 RoPE via signed pair-swap permutation matmul, then repacked
        into the fp8 [32,2,t] DoubleRow layout), the V projection bands,
        and the first half of the output projection threaded through a
        fine-grained PE filler FIFO so ACT never starves.
        Per head: S^T[t,l] via fp8e4 DoubleRow (2x PE rate), P^T =
        exp(scale*S^T) on ACT in bf16 (logits are O(1), no max
        subtraction), attnout^T = [V_h|1].T @ P^T accumulated over t which
        also yields row sums in partition 64; normalize via reciprocal +
        ones-matmul broadcast and stage attnout^T (bf16) to DRAM.
  Out:  out = attnout^T.T @ Wp + bp in fp32, contraction split: pairs 0-3
        pre-accumulated into bf16 SBUF partials during late windows, pairs
        4-7 plus the partial add in the drain.
"""

import sys

sys.path.insert(0, "/opt/trn_rl_repo")

from contextlib import ExitStack

import ml_dtypes
import numpy as np

import concourse.bass as bass
import concourse.mybir as mybir
import concourse.tile as tile
from concourse.bass_utils import run_bass_kernel_spmd

F32 = mybir.dt.float32
F32R = mybir.dt.float32r
BF16 = mybir.dt.bfloat16
F8 = mybir.dt.float8e4
AF = mybir.ActivationFunctionType
MUL = mybir.AluOpType.mult
DR = mybir.MatmulPerfMode.DoubleRow

B, T, D = 4, 2048, 1024
H, HD = 16, 64
P = 128
TQ = 1024  # query rows per core
SCALE = float(D) ** -0.5
ROPE_THETA = 10000.0

_ws_ctr = [0]


def _split_multi_waits(nc):
    """The walrus build in this container accepts at most one sync-wait per
    engine instruction. Hoist all but one wait of each instruction into
    standalone EventSemaphore instructions on the same engine, placed
    immediately before it (engines are in-order, so semantics are identical)."""
    n = 0
    for f in nc.m.functions:
        for blk in f.blocks:
            insts = list(blk.instructions)
            newlist = []
            changed = False
            for inst in insts:
                si = getattr(inst, "sync_info", None)
                waits = list(si.on_wait) if si is not None and si.on_wait else []
                if len(waits) > 1:
                    for w in waits[:-1]:
                        _ws_ctr[0] += 1
                        evs = mybir.InstEventSemaphore(
                            name=f"WSPLIT-{_ws_ctr[0]}", ins=[], outs=[]
                        )
                        evs.engine = inst.engine
                        evs.sync_info = mybir.SyncInfo(on_wait=[w], on_update=[])
                        newlist.append(evs)
                        n += 1
                    inst.sync_info = mybir.SyncInfo(
                        on_wait=[waits[-1]], on_update=list(si.on_update)
                    )
                    changed = True
                newlist.append(inst)
            if changed:
                blk.instructions[:] = newlist
    return n


def _perm_lhsT():
    """lhsT for the rotate-half permutation: out = lhsT.T @ q gives
    out[2i] = -q[2i+1], out[2i+1] = q[2i]."""
    m = np.zeros((P, P), np.float32)
    for i in range(P // 2):
        m[2 * i + 1, 2 * i] = -1.0
        m[2 * i, 2 * i + 1] = 1.0
    return m


def build_nc(split_waits=True, reps=1):
    nc = bass.Bass(trn_type="TRN2", target_bir_lowering=False, debug=False)

    xT = nc.dram_tensor("xT", [D, T], BF16, kind="ExternalInput").ap()
    Wq = nc.dram_tensor("Wq", [D, D], BF16, kind="ExternalInput").ap()
    Wk = nc.dram_tensor("Wk", [D, D], BF16, kind="ExternalInput").ap()
    Wv = nc.dram_tensor("Wv", [D, D], BF16, kind="ExternalInput").ap()
    Wp = nc.dram_tensor("Wp", [D, D], BF16, kind="ExternalInput").ap()
    bpb = nc.dram_tensor("bpb", [P, D], BF16, kind="ExternalInput").ap()
    cosk = nc.dram_tensor("cosk", [P, T], BF16, kind="ExternalInput").ap()
    sink = nc.dram_tensor("sink", [P, T], BF16, kind="ExternalInput").ap()
    out = nc.dram_tensor("out", [TQ, D], F32, kind="ExternalOutput").ap()

    permc = nc.inline_tensor(_perm_lhsT().astype(ml_dtypes.bfloat16), name="permc").ap()
    ones128c = nc.inline_tensor(np.ones((P, 64), np.float32), name="ones128c").ap()
    onescolc = nc.inline_tensor(
        np.ones((P, T // P, 1), ml_dtypes.bfloat16), name="onescolc"
    ).ap()

    # Vd3[h] is a contiguous [128(tp), 16(to), 64] bf16 per-head V block.
    # AOTd holds normalized attnout^T in bf16.
    Vd3 = nc.dram_tensor("Vd3", [H, P, T // P, HD], BF16).ap()
    AOTd = nc.dram_tensor("AOTd", [D // P, P, TQ], BF16).ap()

    xT_r = xT.rearrange("(do dp) t -> dp do t", dp=P)  # [128, 8, 2048]
    Wq_r = Wq.rearrange("(do dp) j -> dp do j", dp=P)
    Wk_r = Wk.rearrange("(do dp) j -> dp do j", dp=P)
    Wv_r = Wv.rearrange("(do dp) j -> dp do j", dp=P)
    Wp_r = Wp.rearrange("(ko kp) j -> kp ko j", kp=P)
    AOTd_r = AOTd.rearrange("ko p l -> p ko l")  # [128, 8, 1024]

    DO = D // P  # 8 contraction tiles
    NT = T // P  # 16 key tiles

    with tile.TileContext(nc) as tc:
      for _rep in range(reps):
        with ExitStack() as top:
            persist = top.enter_context(tc.tile_pool(name="persist", bufs=1))
            ones128 = persist.tile([P, 64], F32R)
            onesva = persist.tile([P, NT, 1], BF16)
            permt = persist.tile([P, P], BF16)
            ck = persist.tile([P, T], BF16)
            sk = persist.tile([P, T], BF16)
            vab = persist.tile([P, H, NT, 65], BF16)

            with tc.tile_pool(name="xpool", bufs=1) as xpool:
                xts = xpool.tile([P, DO, T], BF16)

                # -------- interleaved Q/K projection + attention pipeline ------
                with ExitStack() as pmain:
                    wv1pool = pmain.enter_context(tc.tile_pool(name="w_v1", bufs=1))
                    vpool = pmain.enter_context(tc.tile_pool(name="o_v1", bufs=2))
                    wpool = pmain.enter_context(tc.tile_pool(name="w_qk", bufs=2))
                    tpool = pmain.enter_context(tc.tile_pool(name="t_qk", bufs=2))
                    kqpool = pmain.enter_context(tc.tile_pool(name="kq", bufs=2))
                    k8pool = pmain.enter_context(tc.tile_pool(name="k8", bufs=2))
                    pmm = pmain.enter_context(
                        tc.tile_pool(name="ps_qk", bufs=1, space="PSUM")
                    )
                    pmisc = pmain.enter_context(
                        tc.tile_pool(name="ps_misc", bufs=1, space="PSUM")
                    )

                    def proj_half(ps, w_t, src_col, half):
                        """One contraction sweep over half the free dim."""
                        c0 = src_col * 512 + half * 256
                        for do in range(DO):
                            nc.tensor.matmul(
                                ps[:, half * 256 : half * 256 + 256],
                                w_t[:, do],
                                xts[:, do, c0 : c0 + 256],
                                start=(do == 0),
                                stop=(do == DO - 1),
                            )

                    def rope_tail(ps, dst, dst_col, src_col):
                        raw = tpool.tile([P, 512], BF16, tag="raw")
                        nc.vector.tensor_copy(raw[:], ps[:])
                        pr = pmisc.tile([P, 512], F32, tag="misc")
                        nc.tensor.matmul(pr[:], permt[:], raw[:], start=True, stop=True)
                        t1 = tpool.tile([P, 512], BF16, tag="t1")
                        nc.gpsimd.tensor_tensor(
                            t1[:],
                            raw[:],
                            ck[:, src_col * 512 : (src_col + 1) * 512],
                            MUL,
                        )
                        t2 = tpool.tile([P, 512], BF16, tag="t2")
                        nc.vector.tensor_mul(
                            t2[:], pr[:], sk[:, src_col * 512 : (src_col + 1) * 512]
                        )
                        nc.vector.tensor_add(
                            dst[:, dst_col * 512 : (dst_col + 1) * 512], t1[:], t2[:]
                        )

                    def emit_proj_jo(jo, preload=False, piecewise=False):
                        """Q^T/K^T for head pair jo as a list of small filler
                        closures (two matmul halves + rope tail per 512-col
                        tile, then fp8 [32,2] repack DMAs). Consecutive
                        closures sharing the pmm "ps" bank rely on FIFO
                        adjacency - nothing else allocates that tag between
                        pops."""
                        ktp = kqpool.tile([P, T], F8, tag="ktp")
                        qtp = kqpool.tile([P, TQ], F8, tag="qtp")
                        kp8 = k8pool.tile([P, 2, T], F8, tag="kp8")
                        qp8 = k8pool.tile([P, 2, TQ], F8, tag="qp8")
                        steps = []
                        wref = []
                        state = {}

                        def load_w():
                            wq_t = wpool.tile([P, DO, P], BF16, tag="wq")
                            nc.gpsimd.dma_start(
                                wq_t[:], Wq_r[:, :, jo * P : (jo + 1) * P]
                            )
                            wk_t = wpool.tile([P, DO, P], BF16, tag="wk")
                            nc.gpsimd.dma_start(
                                wk_t[:], Wk_r[:, :, jo * P : (jo + 1) * P]
                            )
                            return wq_t, wk_t

                        if preload:
                            wref.extend(load_w())

                        def half_a(which, col):
                            if not wref:
                                wref.extend(load_w())
                            w_t = wref[0] if which == "q" else wref[1]
                            ps = pmm.tile([P, 512], F32, tag="ps", name="psp")
                            state["ps"] = ps
                            proj_half(ps, w_t, col, 0)

                        def half_b(which, col):
                            w_t = wref[0] if which == "q" else wref[1]
                            ps = state["ps"]
                            proj_half(ps, w_t, col, 1)
                            dst = qtp if which == "q" else ktp
                            rope_tail(ps, dst, col, col)

                        def repack(src, dst, c0, c1):
                            for h in range(2):
                                for i in range(2):
                                    nc.sync.dma_start(
                                        dst[64 * h : 64 * h + 32, i, c0:c1],
                                        src[64 * h + 32 * i : 64 * h + 32 * i + 32,
                                            c0:c1],
                                    )

                        order = [("q", 0), ("k", 0), ("k", 1), ("k", 2), ("k", 3),
                                 ("q", 1)]
                        for which, col in order:
                            steps.append(lambda w=which, c=col: half_a(w, c))
                            steps.append(lambda w=which, c=col: half_b(w, c))
                            if piecewise:
                                if which == "q":
                                    steps.append(
                                        lambda c=col: repack(
                                            qtp, qp8, c * 512, (c + 1) * 512
                                        )
                                    )
                                else:
                                    steps.append(
                                        lambda c=col: repack(
                                            ktp, kp8, c * 512, (c + 1) * 512
                                        )
                                    )
                        if not piecewise:
                            steps.append(lambda: repack(ktp, kp8, 0, T))
                            steps.append(lambda: repack(qtp, qp8, 0, TQ))
                        return kp8, qp8, steps

                    # ------------- windowed attention pipeline ----------------
                    # 16 windows w = (pair p, query half lc). Window w emits
                    # S+exp for (p, lc) while retiring the PV accumulation and
                    # softmax normalization of window w-1, with projection /
                    # V-projection / output-projection steps as PE fillers.
                    vapool = pmain.enter_context(tc.tile_pool(name="va", bufs=2))
                    ptpool = pmain.enter_context(tc.tile_pool(name="pt", bufs=36))
                    smpool = pmain.enter_context(tc.tile_pool(name="sm", bufs=2))
                    vpool0 = pmain.enter_context(tc.tile_pool(name="o_v", bufs=2))
                    wvpool = pmain.enter_context(tc.tile_pool(name="w_v", bufs=1))
                    wppool = pmain.enter_context(tc.tile_pool(name="wp", bufs=1))
                    apool = pmain.enter_context(tc.tile_pool(name="aot", bufs=3))
                    outpool = pmain.enter_context(tc.tile_pool(name="outp", bufs=2))
                    pss = pmain.enter_context(
                        tc.tile_pool(name="ps_s", bufs=4, space="PSUM")
                    )
                    pso = pmain.enter_context(
                        tc.tile_pool(name="ps_o", bufs=1, space="PSUM")
                    )

                    kp80, qp80, steps0 = emit_proj_jo(0, preload=True, piecewise=True)
                    # V weights in three column bands: heads 0-1 first so the
                    # first PV sweep unblocks early, then heads 2-9, 10-15.
                    for q in range(2):
                        nc.sync.dma_start(
                            xts[:, :, q * 256 : (q + 1) * 256],
                            xT_r[:, :, q * 256 : (q + 1) * 256],
                        )
                    nc.sync.dma_start(ck[:], cosk[:])
                    nc.sync.dma_start(sk[:], sink[:])
                    nc.sync.dma_start(permt[:], permc)
                    wv_a = wvpool.tile([P, DO, P], BF16, tag="wva")
                    nc.sync.dma_start(wv_a[:], Wv_r[:, :, 0:128])
                    for q in range(2, 8):
                        nc.sync.dma_start(
                            xts[:, :, q * 256 : (q + 1) * 256],
                            xT_r[:, :, q * 256 : (q + 1) * 256],
                        )
                    nc.sync.dma_start(ones128[:], ones128c.bitcast(F32R))
                    nc.sync.dma_start(onesva[:], onescolc)
                    wv_b = wvpool.tile([P, DO, 512], BF16, tag="bigw", name="wv_b")
                    wv_c = wvpool.tile([P, DO, 384], BF16, tag="bigw", name="wv_c")
                    for hh in range(H):
                        nc.vector.tensor_copy(vab[:, hh, :, 64:65], onesva[:])

                    def v_band(band, to, half):
                        """V projection for one t-tile of a column band.
                        band 0: cols 0:128 (heads 0-1, single shot);
                        band 1: cols 128:640 in two halves;
                        band 2: cols 640:1024 in two halves."""
                        wv, j0, w = [
                            (wv_a, 0, 128), (wv_b, 128, 512), (wv_c, 640 - 128, 384)
                        ][band]
                        j0 = [0, 128, 640][band]
                        if band == 0:
                            cw, c0 = 128, 0
                        else:
                            cw, c0 = w // 2, half * (w // 2)
                        if half == 0 or band == 0:
                            ps = pmm.tile([P, 512], F32, tag="ps", name="psv")
                            state = ps
                            vps[to] = ps
                        ps = vps[to]
                        for do in range(DO):
                            nc.tensor.matmul(
                                ps[:, c0 : c0 + cw],
                                xts[:, do, to * P : (to + 1) * P],
                                wv[:, do, c0 : c0 + cw],
                                start=(do == 0),
                                stop=(do == DO - 1),
                            )
                        if half == 1 or band == 0:
                            h0, nh = j0 // 64, w // 64
                            nc.vector.tensor_copy(
                                vab[:, h0 : h0 + nh, to, 0:64],
                                ps[:, 0:w].rearrange("tp (h e) -> tp h e", e=64),
                            )

                    vps = {}

                    wp_ref = []

                    def load_wp():
                        wp_t = wvpool.tile([P, DO, D], BF16, tag="bigw", name="wp_t")
                        for ko in range(DO):
                            nc.sync.dma_start(wp_t[:, ko], Wp_r[:, ko])
                        bpt = wppool.tile([P, D], BF16, name="bpt")
                        nc.sync.dma_start(bpt[:], bpb[:])
                        wp_ref.extend((wp_t, bpt))

                    o1pool = pmain.enter_context(tc.tile_pool(name="o1", bufs=8))
                    aot_ref = {}
                    o1_ref = {}
                    outps = {}

                    # contraction split: ph0 = pairs 0-3 (copy to o1),
                    # ph2 = pairs 4-6 (in-place add to o1), ph1 = pair 7 +
                    # bias + partial add (drain).
                    KO_RANGE = {0: (0, 4), 2: (4, 7), 1: (7, 8)}

                    def out_load(lt, ph):
                        k0, k1 = KO_RANGE[ph]
                        aot_t = apool.tile([P, 4, P], BF16, tag="aot")
                        nc.sync.dma_start(
                            aot_t[:, 0 : k1 - k0],
                            AOTd_r[:, k0:k1, lt * P : (lt + 1) * P],
                        )
                        aot_ref[lt, ph] = aot_t
                        if ph == 0:
                            o1_ref[lt] = o1pool.tile([P, TQ], BF16, tag="o1", name="o1t")

                    def out_quarter(lt, ph, jc, half):
                        wp_t, bpt = wp_ref
                        aot_t = aot_ref[lt, ph]
                        k0, k1 = KO_RANGE[ph]
                        j0 = jc * 512 + half * 256
                        if half == 0:
                            ps = pmm.tile([P, 512], F32, tag="ps", name="psf")
                            outps[lt] = ps
                        ps = outps[lt]
                        for ko in range(k1 - k0):
                            nc.tensor.matmul(
                                ps[:, half * 256 : half * 256 + 256],
                                aot_t[:, ko],
                                wp_t[:, k0 + ko, j0 : j0 + 256],
                                start=(ko == 0),
                                stop=(ko == k1 - k0 - 1),
                            )
                        if half == 1:
                            o1s = o1_ref[lt][:, jc * 512 : (jc + 1) * 512]
                            if ph == 0:
                                # bias folded into the first partial
                                nc.vector.tensor_add(
                                    o1s, ps[:], bpt[:, jc * 512 : (jc + 1) * 512]
                                )
                            elif ph == 2:
                                nc.vector.tensor_add(o1s, o1s, ps[:])
                            else:
                                ot2 = outpool.tile([P, 512], F32, tag="oto2")
                                nc.vector.tensor_add(ot2[:], ps[:], o1s)
                                nc.sync.dma_start(
                                    out[lt * P : (lt + 1) * P,
                                        jc * 512 : (jc + 1) * 512],
                                    ot2[:],
                                )

                    def out_steps(lt, ph):
                        return [
                            lambda: out_load(lt, ph),
                            lambda: out_quarter(lt, ph, 0, 0),
                            lambda: out_quarter(lt, ph, 0, 1),
                            lambda: out_quarter(lt, ph, 1, 0),
                            lambda: out_quarter(lt, ph, 1, 1),
                        ]

                    vvs_ref = {}

                    def make_pv_norm(w, pts_list):
                        p, lc = divmod(w, 2)
                        pos = [None, None]

                        def pv_step(to):
                            if to == 0:
                                pos[0] = pso.tile([P, 512], F32, tag="po0", name="po0")
                                pos[1] = pso.tile([P, 512], F32, tag="po1", name="po1")
                            for h in range(2):
                                nc.tensor.matmul(
                                    pos[h][0:65, :],
                                    vab[:, 2 * p + h, to, :],
                                    pts_list[to][h][:],
                                    start=(to == 0),
                                    stop=(to == NT - 1),
                                )

                        sm_ref = {}

                        def norm_a(h):
                            # DVE-only half: copy accumulator out of PSUM and
                            # take the row-sum reciprocal (frees pos fast).
                            po_s = smpool.tile([65, 512], F32, tag="pos")
                            nc.vector.tensor_copy(po_s[:], pos[h][0:65, :])
                            rc = smpool.tile([65, 512], F32R, tag="rc")
                            with nc.allow_low_precision(
                                reason="f32r feeds the broadcast matmul"
                            ):
                                nc.vector.reciprocal(rc[64:65, :], po_s[64:65, :])
                            sm_ref[h] = (po_s, rc)

                        def norm_b(h):
                            # PE-broadcast half, deferred into the next window.
                            po_s, rc = sm_ref[h]
                            pb = pmisc.tile(
                                [P, 512], F32, tag="misc", name="pb"
                            )[0:64, :]
                            nc.tensor.matmul(
                                pb[:],
                                ones128[64:65, :],
                                rc[64:65, :],
                                start=True,
                                stop=True,
                            )
                            rbb = smpool.tile([64, 512], F32, tag="rbb")
                            nc.vector.tensor_copy(rbb[:], pb[:])
                            tmpn = smpool.tile([64, 512], BF16, tag="tmpn")
                            nc.vector.tensor_mul(tmpn[:], po_s[0:64, :], rbb[:])
                            nc.sync.dma_start(
                                AOTd[p, 64 * h : 64 * h + 64,
                                     lc * 512 : (lc + 1) * 512],
                                tmpn[:],
                            )

                        return pv_step, norm_a, norm_b

                    fillers = []

                    def pop_fillers(k):
                        n = 0
                        while fillers and n < k:
                            fillers.pop(0)()
                            n += 1

                    def emit_window(w, kp8, qp8, prev, pop_k=1):
                        p, lc = divmod(w, 2)
                        pts_list = []
                        for to in range(NT):
                            pts = []
                            for h in range(2):
                                ps = pss.tile([P, 512], F32, tag="pss")
                                nc.tensor.matmul(
                                    ps[:],
                                    kp8[64 * h : 64 * h + 32, :,
                                        to * P : (to + 1) * P],
                                    qp8[64 * h : 64 * h + 32, :,
                                        lc * 512 : (lc + 1) * 512],
                                    start=True,
                                    stop=True,
                                    perf_mode=DR,
                                )
                                pt = ptpool.tile([P, 512], BF16, tag="pt")
                                nc.scalar.activation(pt[:], ps[:], AF.Exp, scale=SCALE)
                                pts.append(pt)
                            pts_list.append(pts)
                            if prev is not None:
                                prev[0](to)
                            pop_fillers(pop_k if len(fillers) < 12 else pop_k + 1)
                        if prev is not None:
                            prev[1](0)
                            prev[1](1)
                        nprev = make_pv_norm(w, pts_list)
                        if prev is not None:
                            fillers.insert(0, lambda: (prev[2](0), prev[2](1)))
                        return nprev

                    # jo0's first two tiles (q cols 0:512, k cols 0:512) must
                    # fully precede the first S consumer (in-order PE); the
                    # rest of jo0 flows through the filler FIFO ahead of the
                    # to-positions that consume each K tile.
                    for s in steps0[:6]:
                        s()
                    fillers.extend(steps0[6:])

                    prev = None
                    kp8c, qp8c = kp80, qp80
                    nkp8 = nqp8 = None
                    for w in range(16):
                        p, lc = divmod(w, 2)
                        if lc == 0 and p + 1 < DO:
                            nkp8, nqp8, nsteps = emit_proj_jo(p + 1)
                            fillers.extend(nsteps)
                        if w == 0:
                            fillers.extend(
                                (lambda to=to: v_band(0, to, 0)) for to in range(NT)
                            )
                        elif w in (1, 2):
                            if w == 1:
                                fillers.append(
                                    lambda: nc.sync.dma_start(
                                        wv_b[:], Wv_r[:, :, 128:640]
                                    )
                                )
                            rng = range(0, 10) if w == 1 else range(10, NT)
                            fillers.extend(
                                (lambda to=to, h=h: v_band(1, to, h))
                                for to in rng
                                for h in range(2)
                            )
                        elif w in (3, 4):
                            if w == 3:
                                fillers.append(
                                    lambda: nc.sync.dma_start(
                                        wv_c[:], Wv_r[:, :, 640:1024]
                                    )
                                )
                            rng = range(0, 10) if w == 3 else range(10, NT)
                            fillers.extend(
                                (lambda to=to, h=h: v_band(2, to, h))
                                for to in rng
                                for h in range(2)
                            )
                        if w == 9:
                            load_wp()
                        if w == 10:
                            for lt in range(4):
                                fillers.extend(out_steps(lt, 0))
                        if w == 12:
                            for lt in range(4, TQ // P):
                                fillers.extend(out_steps(lt, 0))
                        if w == 14:
                            for lt in range(4):
                                fillers.extend(out_steps(lt, 2))
                        if w == 15:
                            for lt in range(4, TQ // P):
                                fillers.extend(out_steps(lt, 2))
                        prev = emit_window(
                            w, kp8c, qp8c, prev, pop_k=(3 if w == 0 else 1)
                        )
                        if lc == 1 and nkp8 is not None:
                            kp8c, qp8c = nkp8, nqp8
                            nkp8 = nqp8 = None

                    # drain: final window's PV + norms, then output projection
                    pv_f, norm_a_f, norm_b_f = prev
                    for lt in range(4):
                        fillers.extend(out_steps(lt, 1))
                    for to in range(NT):
                        pv_f(to)
                        pop_fillers(3)
                    norm_a_f(0)
                    norm_a_f(1)
                    pop_fillers(len(fillers))
                    norm_b_f(0)
                    norm_b_f(1)
                    for lt in range(4, TQ // P):
                        out_load(lt, 1)
                    for lt in range(4, TQ // P):
                        for s in out_steps(lt, 1)[1:]:
                            s()
                    pop_fillers(len(fillers))

    if split_waits:
        _split_multi_waits(nc)
    return nc


def _rope_tables():
    inv = 1.0 / (ROPE_THETA ** (np.arange(0, HD, 2, dtype=np.float32) / HD))
    t = np.arange(T, dtype=np.float32)
    freqs = np.einsum("i,j->ij", t, inv)  # [T, 32]
    freqs = np.repeat(freqs, 2, axis=-1)  # [T, 64]
    cosT = np.cos(freqs).T  # [64, T]
    sinT = np.sin(freqs).T
    cosk = np.tile(cosT, (2, 1)).astype(np.float32)  # [128, T]
    sink = np.tile(sinT, (2, 1)).astype(np.float32)
    return np.ascontiguousarray(cosk), np.ascontiguousarray(sink)


_NC_CACHE = {}


def make_in_maps(x, Wq, Wk, Wv, Wp, bp):
    cosk, sink = _rope_tables()
    bf = ml_dtypes.bfloat16
    bpb = np.ascontiguousarray(np.tile(np.asarray(bp, np.float32)[None, :].astype(bf), (P, 1)))
    Wq = np.ascontiguousarray(np.asarray(Wq, np.float32).astype(bf))
    Wk = np.ascontiguousarray(np.asarray(Wk, np.float32).astype(bf))
    Wv = np.ascontiguousarray(np.asarray(Wv, np.float32).astype(bf))
    Wp = np.ascontiguousarray(np.asarray(Wp, np.float32).astype(bf))
    in_maps = []
    for c in range(8):
        b, qh = c // 2, c % 2
        xT = np.asarray(x[b], np.float32).T.astype(bf)  # [D, T]
        roll = qh * TQ
        in_maps.append(
            {
                "xT": np.ascontiguousarray(np.roll(xT, -roll, axis=1)),
                "Wq": Wq,
                "Wk": Wk,
                "Wv": Wv,
                "Wp": Wp,
                "bpb": bpb,
                "cosk": np.ascontiguousarray(np.roll(cosk, -roll, axis=1)).astype(bf),
                "sink": np.ascontiguousarray(np.roll(sink, -roll, axis=1)).astype(bf),
            }
        )
    return in_maps


def kernel(x, h, w, Wq, Wk, Wv, Wp, bp, _trace=False, **trace_kwargs):
    x = np.asarray(x, np.float32)
    in_maps = make_in_maps(x, Wq, Wk, Wv, Wp, bp)
    if "nc" not in _NC_CACHE:
        _NC_CACHE["nc"] = build_nc()
    nc = _NC_CACHE["nc"]
    res = run_bass_kernel_spmd(
        nc, in_maps, list(range(8)), trace=_trace, **trace_kwargs
    )
    out = np.empty((B, T, D), np.float32)
    for c in range(8):
        b, qh = c // 2, c % 2
        out[b, qh * TQ : (qh + 1) * TQ, :] = res.results[c]["out"]
    kernel.last_result = res
    return out
